# revision 10
# baseline (speedup 1.0000x reference)
"""Trainium2 Bass kernel: attention-LSTM decoder (nn_Attention_74698071212133).

Sharding: data-parallel over batch across 8 NeuronCores (64 rows each), weights
replicated.  Each core further splits its 64 batch rows into NCHUNK independent
chunks so the Tile scheduler can overlap engines across the sequential scan.

Per-core algorithm (batch b, T=64 enc positions, H=512, C=38, S=26 steps):
  H_projT[h, (b,t)] = w_i2h @ encT + b_h2h     (once, bf16, SBUF-resident)
  per step s:
    hpT  = w_h2h @ hT                               (PE)
    a    = H_projT + broadcast_t(hpT)               (DVE, bf16 2x)
    th   = tanh(a)                                  (ACT, bf16)
    e    = w_score . th       -> psum[bt-part, 16]  (PE, lhsT=th slices)
    e_tr = PE-transpose(e)    -> [16, (2b x 64t)]
    alpha= softmax_t(e_tr)                          (DVE/ACT, fp32)
    a_col= PE-transpose(alpha)-> [bt-part, 16]      -> block-diag lhsT
    ctx  = alphaT @ enc       (block-diag lhsT)     (PE)
    ctxT = PE-transpose(ctx)
    gT   = W_x @ [ctxT; ohT; 1] + W_h @ hT          (PE, bias via ones-row)
    lstm elementwise with sigmoid(x)=0.5*tanh(x/2)+0.5  (ACT+DVE)
    probs= hT.T @ w_genT + b_gen  -> DRAM           (PE + DMA)
"""

import sys

sys.path.insert(0, "/opt/trn_rl_repo")

import os
import numpy as np
import ml_dtypes

import concourse.bass as bass
import concourse.mybir as mybir
import concourse.tile as tile
from concourse import bacc
from concourse.bass_utils import run_bass_kernel_spmd

BF = ml_dtypes.bfloat16
F32 = mybir.dt.float32
BF16 = mybir.dt.bfloat16
AF = mybir.ActivationFunctionType
ALU = mybir.AluOpType

DEBUG = bool(int(os.environ.get("KDEBUG", "0")))

# Problem constants
B, T, D, H, C, S = 512, 64, 512, 512, 38, 26
NCORES = 8
BCORE = B // NCORES  # 64
NCHUNK = 2
G4 = 4 * H  # 2048
HK = H // 128  # 4 h-tiles


def _tile128(a):
    """[R, N] with R = r*128 -> [128, r*N] col-block layout (block k = rows 128k..)."""
    r = a.shape[0] // 128
    return np.ascontiguousarray(
        a.reshape(r, 128, a.shape[1]).transpose(1, 0, 2).reshape(128, -1)
    )


def build_nc(steps=S, nchunk=NCHUNK):
    bc = BCORE // nchunk  # batch per chunk
    bt = bc * T  # flattened (b, t) per chunk, b-major
    nbt = bt // 128  # 128-row bt tiles per chunk (16)
    nb5 = bt // 512  # 512-col bt chunks (4)

    nc = bacc.Bacc()
    dp = nc.declare_dram_parameter
    # Per-core tensors (pre-tiled on host into [128, cols] SBUF images)
    d_enc = dp("enc_sb", [nchunk, 128, nbt * 512], BF16, isOutput=False)
    d_encT = dp("encT_sb", [nchunk, 128, HK * bt], BF16, isOutput=False)
    d_oh = dp("ohT_sb", [C + 1, steps * BCORE], BF16, isOutput=False)
    # Replicated weights
    d_wi2h = dp("w_i2hT", [128, HK * H], BF16, isOutput=False)
    d_wh2h = dp("w_h2hT", [128, HK * H], BF16, isOutput=False)
    d_wsc = dp("w_scoreT", [128, HK], BF16, isOutput=False)
    d_wctx = dp("w_ctxT", [128, HK * G4], BF16, isOutput=False)
    d_whh = dp("w_hhT", [128, HK * G4], BF16, isOutput=False)
    d_woh = dp("w_ohT", [C + 1, G4], BF16, isOutput=False)
    d_wgen = dp("w_genT", [128, HK * C], BF16, isOutput=False)
    d_bgen = dp("b_gen", [1, C], BF16, isOutput=False)
    d_bh2h = dp("b_h2hT", [128, HK], F32, isOutput=False)
    d_idf = dp("id_f32", [128, 128], F32, isOutput=False)
    d_idb = dp("id_bf16", [128, 128], BF16, isOutput=False)
    d_ones = dp("ones_row", [1, BCORE], BF16, isOutput=False)
    d_out = dp("probs", [BCORE, steps, C], F32, isOutput=True)
    dbg = {}
    if DEBUG:
        dbg["hproj"] = dp("dbg_hproj", [128, 512], BF16, isOutput=True)
        dbg["a"] = dp("dbg_a", [128, 512], BF16, isOutput=True)
        dbg["th"] = dp("dbg_th", [128, 512], BF16, isOutput=True)
        dbg["e2"] = dp("dbg_e2", [128, 16], F32, isOutput=True)
        dbg["al"] = dp("dbg_al", [16, 128], F32, isOutput=True)
        dbg["acol"] = dp("dbg_acol", [128, 16], BF16, isOutput=True)
        dbg["ad"] = dp("dbg_ad", [128, 32], BF16, isOutput=True)
        dbg["ctxT"] = dp("dbg_ctxT", [128, 128], BF16, isOutput=True)
        dbg["tifo"] = dp("dbg_tifo", [128, 96], F32, isOutput=True)
        dbg["tg"] = dp("dbg_tg", [128, 32], F32, isOutput=True)
        dbg["hT"] = dp("dbg_hT", [128, 128], BF16, isOutput=True)
        dbg["cT"] = dp("dbg_cT", [128, 128], F32, isOutput=True)

    with tile.TileContext(nc) as tc:
        with (
            tc.tile_pool(name="consts", bufs=1) as pc,
            tc.tile_pool(name="persist", bufs=1) as pp,
        ):
            # ---- load constants ----
            def cload(dram, shape, dt):
                t_ = pc.tile(list(shape), dt, name=dram.tensor.name + "_sb")
                nc.sync.dma_start(t_[:], dram)
                return t_

            w_i2h = cload(d_wi2h[:], [128, HK * H], BF16)
            w_h2h = cload(d_wh2h[:], [128, HK * H], BF16)
            w_sc = cload(d_wsc[:], [128, HK], BF16)
            w_ctx = cload(d_wctx[:], [128, HK * G4], BF16)
            w_hh = cload(d_whh[:], [128, HK * G4], BF16)
            w_oh = cload(d_woh[:], [C + 1, G4], BF16)
            w_gen = cload(d_wgen[:], [128, HK * C], BF16)
            b_gen = cload(d_bgen[:], [1, C], BF16)
            b_h2h = cload(d_bh2h[:], [128, HK], F32)
            id_f = cload(d_idf[:], [128, 128], F32)
            id_b = cload(d_idb[:], [128, 128], BF16)
            ones = cload(d_ones[:], [1, BCORE], BF16)
            ohT = cload(d_oh[:], [C + 1, steps * BCORE], BF16)

            # ---- persistent per-chunk state ----
            enc_sb, hproj, hT, cT, ad = [], [], [], [], []
            for c in range(nchunk):
                e_ = pp.tile([128, nbt * 512], BF16, tag=f"enc{c}")
                for q in range(4):
                    w = nbt * 512 // 4
                    nc.sync.dma_start(
                        e_[:, q * w : (q + 1) * w], d_enc[c, :, q * w : (q + 1) * w]
                    )
                enc_sb.append(e_)
                hproj.append(pp.tile([128, HK * bt], BF16, tag=f"hproj{c}", name=f"hproj{c}"))
                h_ = pp.tile([128, HK * bc], BF16, tag=f"hT{c}")
                c_ = pp.tile([128, HK * bc], F32, tag=f"cT{c}")
                nc.vector.memset(h_[:], 0.0)
                nc.vector.memset(c_[:], 0.0)
                hT.append(h_)
                cT.append(c_)
                a_ = pp.tile([128, 2 * (bc // 2)], BF16, tag=f"ad{c}")
                nc.vector.memset(a_[:], 0.0)
                ad.append(a_)

            # ---- init: H_projT = w_i2h @ encT + b_h2h ----
            # encT lives in the persistent pool: recycling its address range
            # would hand every later first-writer a WAW dep on all 8 input
            # DMA queues, exceeding the per-instruction sync-wait limit.
            with (
                tc.tile_pool(name="initps", bufs=4, space="PSUM") as pips,
            ):
                for c in range(nchunk):
                    et = pp.tile([128, HK * bt], BF16, tag=f"encT{c}", name=f"encT{c}")
                    for q in range(4):
                        w = HK * bt // 4
                        nc.sync.dma_start(
                            et[:, q * w : (q + 1) * w], d_encT[c, :, q * w : (q + 1) * w]
                        )
                    for m in range(HK):
                        for n in range(bt // 512):
                            ps = pips.tile([128, 512], F32, tag="initp")
                            for k in range(HK):
                                nc.tensor.matmul(
                                    ps[:],
                                    w_i2h[:, k * H + 128 * m : k * H + 128 * m + 128],
                                    et[:, k * bt + 512 * n : k * bt + 512 * n + 512],
                                    start=(k == 0),
                                    stop=(k == HK - 1),
                                )
                            nc.scalar.activation(
                                hproj[c][:, m * bt + 512 * n : m * bt + 512 * n + 512],
                                ps[:],
                                AF.Identity,
                                bias=b_h2h[:, m : m + 1],
                            )

            # ---- decode steps ----
            with (
                tc.tile_pool(name="work", bufs=6) as pw,
                tc.tile_pool(name="small", bufs=4) as psm,
                tc.tile_pool(name="ps_hp", bufs=1, space="PSUM") as ps_hp,
                tc.tile_pool(name="ps_e", bufs=1, space="PSUM") as ps_e,
                tc.tile_pool(name="ps_tr", bufs=2, space="PSUM") as ps_tr,
                tc.tile_pool(name="ps_ctx", bufs=1, space="PSUM") as ps_ctx,
                tc.tile_pool(name="ps_g", bufs=2, space="PSUM") as ps_g,
            ):
                for s in range(steps):
                    for c in range(nchunk):
                        step_body(
                            nc, tc, s, c, bc, bt, steps,
                            pw, psm, ps_hp, ps_e, ps_tr, ps_ctx, ps_g,
                            enc_sb[c], hproj[c], hT[c], cT[c], ad[c],
                            w_h2h, w_sc, w_ctx, w_hh, w_oh, w_gen, b_gen,
                            ohT, ones, id_f, id_b, d_out, dbg,
                        )
    if not nc.is_finalized():
        nc.finalize()
    return nc


def step_body(
    nc, tc, s, c, bc, bt, steps,
    pw, psm, ps_hp, ps_e, ps_tr, ps_ctx, ps_g,
    enc_sb, hproj, hT, cT, ad,
    w_h2h, w_sc, w_ctx, w_hh, w_oh, w_gen, b_gen,
    ohT, ones, id_f, id_b, d_out, dbg,
):
    dodbg = bool(dbg) and s == 0 and c == 0
    def dump(key, ap):
        if dodbg:
            nc.sync.dma_start(dbg[key][:], ap)
    nb2 = bc // 2  # bt tiles of 128 rows (16 when bc=32)
    nb5 = bt // 512  # 512-wide bt chunks (4)

    # -- snapshot h (it is overwritten per-k during the LSTM update below) --
    h_use = psm.tile([128, HK * bc], BF16, tag="h_use")
    nc.vector.tensor_copy(h_use[:], hT[:])

    # -- hp = w_h2h @ h : psum [128, HK*bc], then duplicate-x2 copy to bf16 --
    php = ps_hp.tile([128, HK * bc], F32, tag="hp")
    for m in range(HK):
        for k in range(HK):
            nc.tensor.matmul(
                php[:, m * bc : (m + 1) * bc],
                w_h2h[:, k * H + 128 * m : k * H + 128 * m + 128],
                h_use[:, k * bc : (k + 1) * bc],
                start=(k == 0),
                stop=(k == HK - 1),
            )
    hp2 = psm.tile([128, HK * bc * 2], BF16, tag="hp2")
    nc.vector.tensor_copy(
        hp2[:].rearrange("p (x two) -> p x two", two=2),
        php[:].unsqueeze(2).broadcast_to([128, HK * bc, 2]),
    )
    hp2v = hp2[:].rearrange("p (m b two) -> p m b two", m=4, two=2)

    # -- attention scores: a = hproj + hp, th = tanh(a), e = w_score . th --
    pe2 = ps_e.tile([128, bt // 128], F32, tag="e2")
    for n in range(nb5):
        ths = []
        for k in range(4):
            sl = hproj[:, k * bt + 512 * n : k * bt + 512 * n + 512]
            a = pw.tile([128, 512], BF16, tag="a")
            nc.vector.tensor_add(
                a[:].rearrange("p (b t2 two) -> p b t2 two", b=8, two=2),
                sl.rearrange("p (b t2 two) -> p b t2 two", b=8, two=2),
                hp2v[:, k, 8 * n : 8 * n + 8, :].unsqueeze(2).broadcast_to([128, 8, 32, 2]),
            )
            th = pw.tile([128, 512], BF16, tag="th")
            nc.scalar.activation(th[:], a[:], AF.Tanh)
            if n == 0 and k == 0:
                dump("hproj", hproj[:, 0:512])
                dump("a", a[:])
                dump("th", th[:])
            ths.append(th)
        for cc in range(4):
            j = 4 * n + cc
            for k in range(4):
                nc.tensor.matmul(
                    pe2[:, j : j + 1],
                    ths[k][:, 128 * cc : 128 * cc + 128],
                    w_sc[:, k : k + 1],
                    start=(k == 0),
                    stop=(k == 3),
                    skip_group_check=True,
                )
    e2 = psm.tile([128, bt // 128], F32, tag="e2sb")
    nc.vector.tensor_copy(e2[:], pe2[:])
    dump("e2", e2[:])

    # -- transpose e to [16, (2b x 64t)], softmax over t --
    ptr = ps_tr.tile([bt // 128, 128], F32, tag="tr")
    nc.tensor.transpose(ptr[:], e2[:], id_f[:])
    etr = psm.tile([bt // 128, 128], F32, tag="etrsb")
    nc.vector.tensor_copy(etr[:], ptr[:])

    ex = psm.tile([bt // 128, 128], F32, tag="ex")
    nc.scalar.activation(ex[:], etr[:], AF.Exp)
    ssum = psm.tile([bt // 128, 2], F32, tag="ssum")
    nc.vector.reduce_sum(
        ssum[:], ex[:].rearrange("p (b t) -> p b t", b=2), axis=mybir.AxisListType.X
    )
    rinv = psm.tile([bt // 128, 2], F32, tag="rinv")
    nc.vector.reciprocal(rinv[:], ssum[:])
    al = psm.tile([bt // 128, 128], F32, tag="al")
    nc.vector.tensor_mul(
        al[:].rearrange("p (b t) -> p b t", b=2),
        ex[:].rearrange("p (b t) -> p b t", b=2),
        rinv[:].unsqueeze(2).broadcast_to([bt // 128, 2, 64]),
    )
    dump("al", al[:])

    # -- alpha back to bt-partitions, build block-diag lhsT --
    pac = ps_tr.tile([128, bt // 128], F32, tag="tr")
    nc.tensor.transpose(pac[:], al[:], id_f[0 : bt // 128, 0 : bt // 128])
    acol = psm.tile([128, bt // 128], BF16, tag="acolsb")
    nc.vector.tensor_copy(acol[:], pac[:])
    dump("acol", acol[:])
    adv = ad[:].rearrange("p (i two) -> p i two", two=2)
    for j in range(2):
        nc.vector.tensor_copy(adv[64 * j : 64 * j + 64, :, j], acol[64 * j : 64 * j + 64, :])
    dump("ad", ad[:])

    # -- ctxT[d, b] directly: lhsT = enc tile (d -> out partitions),
    #    rhs = block-diag alpha col-pair (b -> out free cols) --
    pctxT = ps_ctx.tile([128, HK * bc], F32, tag="ctxT_ps")
    for m in range(HK):
        for i in range(bc // 2):
            nc.tensor.matmul(
                pctxT[:, m * bc + 2 * i : m * bc + 2 * i + 2],
                enc_sb[:, 512 * i + 128 * m : 512 * i + 128 * m + 128],
                ad[:, 2 * i : 2 * i + 2],
                start=True,
                stop=True,
            )
    ctxT = psm.tile([128, HK * bc], BF16, tag="ctxT")
    nc.vector.tensor_copy(ctxT[:], pctxT[:])
    dump("ctxT", ctxT[:])

    # -- gates (transposed, [i|f|o|g] col-blocks per h-tile k) + LSTM update --
    ohsl = ohT[:, s * BCORE + c * bc : s * BCORE + (c + 1) * bc]
    for k in range(HK):
        pg = ps_g.tile([128, 4 * bc], F32, tag="g")
        for gi, gate in enumerate((0, 1, 3, 2)):  # cols = [i, f, o, g]
            m = 4 * gate + k
            col = pg[:, gi * bc : (gi + 1) * bc]
            for kk in range(HK):
                nc.tensor.matmul(
                    col,
                    w_ctx[:, kk * G4 + 128 * m : kk * G4 + 128 * m + 128],
                    ctxT[:, kk * bc : (kk + 1) * bc],
                    start=(kk == 0),
                    stop=False,
                    skip_group_check=True,
                )
            nc.tensor.matmul(
                col, w_oh[:, 128 * m : 128 * m + 128], ohsl,
                start=False, stop=False, skip_group_check=True,
            )
            for kk in range(HK):
                nc.tensor.matmul(
                    col,
                    w_hh[:, kk * G4 + 128 * m : kk * G4 + 128 * m + 128],
                    h_use[:, kk * bc : (kk + 1) * bc],
                    start=False,
                    stop=(kk == HK - 1),
                    skip_group_check=True,
                )
        tifo = psm.tile([128, 3 * bc], F32, tag="tifo")
        nc.scalar.activation(tifo[:], pg[:, 0 : 3 * bc], AF.Tanh, scale=0.5)
        tg = psm.tile([128, bc], F32, tag="tg")
        nc.scalar.activation(tg[:], pg[:, 3 * bc : 4 * bc], AF.Tanh)
        if k == 0:
            dump("tifo", tifo[:])
            dump("tg", tg[:])
        sifo = psm.tile([128, 3 * bc], F32, tag="sifo")
        nc.vector.tensor_scalar(sifo[:], tifo[:], 0.5, 0.5, ALU.mult, ALU.add)
        m1 = psm.tile([128, bc], F32, tag="m1")
        nc.vector.tensor_mul(m1[:], sifo[:, bc : 2 * bc], cT[:, k * bc : (k + 1) * bc])
        m2 = psm.tile([128, bc], F32, tag="m2")
        nc.vector.tensor_mul(m2[:], sifo[:, 0:bc], tg[:])
        nc.vector.tensor_add(cT[:, k * bc : (k + 1) * bc], m1[:], m2[:])
        tc_ = psm.tile([128, bc], F32, tag="tc")
        nc.scalar.activation(tc_[:], cT[:, k * bc : (k + 1) * bc], AF.Tanh)
        nc.vector.tensor_mul(hT[:, k * bc : (k + 1) * bc], sifo[:, 2 * bc : 3 * bc], tc_[:])

    dump("hT", hT[:])
    dump("cT", cT[:])

    # -- probs = hT.T @ w_genT + b_gen -> DRAM --
    pp_ = ps_e.tile([bc, C], F32, tag="e2")
    for k in range(HK):
        nc.tensor.matmul(
            pp_[:],
            hT[:, k * bc : (k + 1) * bc],
            w_gen[:, k * C : (k + 1) * C],
            start=(k == 0),
            stop=False,
            skip_group_check=True,
        )
    nc.tensor.matmul(
        pp_[:], ones[0:1, 0:bc], b_gen[:], start=False, stop=True, skip_group_check=True
    )
    po = psm.tile([bc, C], F32, tag="po")
    nc.vector.tensor_copy(po[:], pp_[:])
    nc.sync.dma_start(d_out[c * bc : (c + 1) * bc, s, :], po[:])


# ------------------------- host side -------------------------


def prep_inputs(encoder_output, text, w_i2h, w_h2h, b_h2h, w_score, w_ih, w_hh,
                b_ih, b_hh, w_gen, b_gen, steps=S, nchunk=NCHUNK):
    """Build per-core input maps (numpy only)."""
    bc = BCORE // nchunk
    bt = bc * T
    enc = np.asarray(encoder_output, np.float32)
    text = np.asarray(text)

    wid = {}
    wid["w_i2hT"] = _tile128(np.asarray(w_i2h, np.float32).T.astype(BF))  # [D,H] tiles
    wid["w_h2hT"] = _tile128(np.asarray(w_h2h, np.float32).T.astype(BF))
    wid["w_scoreT"] = _tile128(np.asarray(w_score, np.float32).reshape(H, 1).astype(BF))
    wid["w_ctxT"] = _tile128(np.asarray(w_ih, np.float32)[:, :D].T.astype(BF))
    wid["w_hhT"] = _tile128(np.asarray(w_hh, np.float32).T.astype(BF))
    woh = np.zeros((C + 1, G4), BF)
    woh[:C] = np.asarray(w_ih, np.float32)[:, D:].T.astype(BF)
    woh[C] = (np.asarray(b_ih, np.float32) + np.asarray(b_hh, np.float32)).astype(BF)
    wid["w_ohT"] = woh
    wid["w_genT"] = _tile128(np.asarray(w_gen, np.float32).T.astype(BF))
    wid["b_gen"] = np.asarray(b_gen, np.float32).reshape(1, C).astype(BF)
    wid["b_h2hT"] = np.ascontiguousarray(
        np.asarray(b_h2h, np.float32).reshape(HK, 128).T
    )
    wid["id_f32"] = np.eye(128, dtype=np.float32)
    wid["id_bf16"] = np.eye(128, dtype=np.float32).astype(BF)
    wid["ones_row"] = np.ones((1, BCORE), BF)

    in_maps = []
    for core in range(NCORES):
        rows = slice(core * BCORE, (core + 1) * BCORE)
        ec = enc[rows]  # [64, T, D]
        enc_sb = np.zeros((nchunk, 128, (bt // 128) * 512), BF)
        encT_sb = np.zeros((nchunk, 128, HK * bt), BF)
        for c in range(nchunk):
            flat = ec[c * bc : (c + 1) * bc].reshape(bt, D)  # b-major (b,t) rows
            enc_sb[c] = _tile128(flat.astype(BF))
            encT_sb[c] = _tile128(np.ascontiguousarray(flat.T).astype(BF))
        oh = np.zeros((C + 1, steps * BCORE), BF)
        tx = text[rows]  # [64, S]
        for s in range(steps):
            oh[tx[:, s].astype(np.int64), s * BCORE + np.arange(BCORE)] = 1.0
        oh[C] = 1.0
        m = dict(wid)
        m["enc_sb"] = enc_sb
        m["encT_sb"] = encT_sb
        m["ohT_sb"] = oh
        in_maps.append(m)
    return in_maps


_NC_CACHE = {}


def get_nc(steps=S, nchunk=NCHUNK):
    key = (steps, nchunk)
    if key not in _NC_CACHE:
        _NC_CACHE[key] = build_nc(steps, nchunk)
    return _NC_CACHE[key]


def run(inputs, steps=S, nchunk=NCHUNK, trace=False):
    nc = get_nc(steps, nchunk)
    in_maps = prep_inputs(**inputs, steps=steps, nchunk=nchunk)
    res = run_bass_kernel_spmd(nc, in_maps, list(range(NCORES)), trace=trace)
    out = np.concatenate([res.results[i]["probs"] for i in range(NCORES)], axis=0)
    return out.astype(np.float32), res


def kernel(**inputs):
    out, _ = run(inputs)
    return out


# revision 12
# speedup vs baseline: 1.0698x; 1.0698x over previous
"""Trainium2 Bass kernel: attention-LSTM decoder (nn_Attention_74698071212133).

Sharding: data-parallel over batch across 8 NeuronCores (64 rows each), weights
replicated.  Each core splits its 64 rows into NCHUNK chunks whose attention
phases pipeline against each other; the LSTM/gates phase is fused across chunks
(wider matmuls, N=64) since the recurrence joins there anyway.

Per-core, per step s (batch b=32/chunk, T=64, H=512, C=38):
  h_use = snapshot(hT)                         (DVE)
  hpT   = w_h2h @ h_use       [fused chunks]   (PE)
  per chunk: a = H_projT + bcast_t(hpT)        (DVE, bf16 2x, 2048-wide)
             th = tanh(a)                      (ACT)
             e  = w_score . th -> psum[bt,16]  (PE, lhsT=th slices)
             softmax via PE transpose -> alpha (PE/DVE/ACT, fp32)
             ctxT = enc.T @ alpha-blockdiag    (PE, direct [d, b] layout)
  gatesT = W[ctxT; oh; 1; h_use] [fused]       (PE, K=128-padded onehots)
  lstm elementwise, sigmoid via 0.5*tanh(x/2)+0.5  (ACT+DVE)
  probs = hT.T @ w_genT + b_gen -> DRAM        (PE + DMA)
"""

import sys

sys.path.insert(0, "/opt/trn_rl_repo")

import numpy as np
import ml_dtypes

import concourse.bass as bass
import concourse.mybir as mybir
import concourse.tile as tile
from concourse import bacc
from concourse.bass_utils import run_bass_kernel_spmd

BF = ml_dtypes.bfloat16
F32 = mybir.dt.float32
BF16 = mybir.dt.bfloat16
AF = mybir.ActivationFunctionType
ALU = mybir.AluOpType

# Problem constants
B, T, D, H, C, S = 512, 64, 512, 512, 38, 26
NCORES = 8
BCORE = B // NCORES  # 64
NCHUNK = 2
G4 = 4 * H  # 2048
HK = H // 128  # 4 h-tiles


def _tile128(a):
    """[R, N] with R = r*128 -> [128, r*N] col-block layout (block k = rows 128k..)."""
    r = a.shape[0] // 128
    return np.ascontiguousarray(
        a.reshape(r, 128, a.shape[1]).transpose(1, 0, 2).reshape(128, -1)
    )


def build_nc(steps=S, nchunk=NCHUNK):
    bc = BCORE // nchunk  # batch per chunk
    bt = bc * T  # flattened (b, t) per chunk, b-major
    nbt = bt // 128  # 128-row bt tiles per chunk

    nc = bacc.Bacc()
    dp = nc.declare_dram_parameter
    # Per-core tensors (pre-tiled on host into [128, cols] SBUF images)
    d_enc = dp("enc_sb", [nchunk, 128, nbt * 512], BF16, isOutput=False)
    d_encT = dp("encT_sb", [nchunk, 128, HK * bt], BF16, isOutput=False)
    d_oh = dp("ohT_sb", [128, steps * BCORE], BF16, isOutput=False)
    # Replicated weights
    d_wi2h = dp("w_i2hT", [128, HK * H], BF16, isOutput=False)
    d_wh2h = dp("w_h2hT", [128, HK * H], BF16, isOutput=False)
    d_wsc = dp("w_scoreT", [128, HK], BF16, isOutput=False)
    d_wctx = dp("w_ctxT", [128, HK * G4], BF16, isOutput=False)
    d_whh = dp("w_hhT", [128, HK * G4], BF16, isOutput=False)
    d_woh = dp("w_ohT", [128, G4], BF16, isOutput=False)
    d_wgen = dp("w_genT", [128, HK * C], BF16, isOutput=False)
    d_bgen = dp("b_gen", [1, C], BF16, isOutput=False)
    d_bh2h = dp("b_h2hT", [128, HK], F32, isOutput=False)
    d_idf = dp("id_f32", [128, 128], F32, isOutput=False)
    d_ones = dp("ones_row", [1, BCORE], BF16, isOutput=False)
    d_out = dp("probs", [BCORE, steps, C], F32, isOutput=True)

    with tile.TileContext(nc) as tc:
        with (
            tc.tile_pool(name="consts", bufs=1) as pc,
            tc.tile_pool(name="persist", bufs=1) as pp,
        ):
            # ---- load constants ----
            def cload(dram, shape, dt):
                t_ = pc.tile(list(shape), dt, name=dram.tensor.name + "_sb")
                nc.sync.dma_start(t_[:], dram)
                return t_

            w_i2h = cload(d_wi2h[:], [128, HK * H], BF16)
            w_h2h = cload(d_wh2h[:], [128, HK * H], BF16)
            w_sc = cload(d_wsc[:], [128, HK], BF16)
            w_ctx = cload(d_wctx[:], [128, HK * G4], BF16)
            w_hh = cload(d_whh[:], [128, HK * G4], BF16)
            w_oh = cload(d_woh[:], [128, G4], BF16)
            w_gen = cload(d_wgen[:], [128, HK * C], BF16)
            b_gen = cload(d_bgen[:], [1, C], BF16)
            b_h2h = cload(d_bh2h[:], [128, HK], F32)
            id_f = cload(d_idf[:], [128, 128], F32)
            ones = cload(d_ones[:], [1, BCORE], BF16)
            ohT = cload(d_oh[:], [128, steps * BCORE], BF16)

            # ---- persistent state (fused layout: col-block k is BCORE wide,
            #      [chunk0 bc | chunk1 bc]) ----
            hT = pp.tile([128, HK * BCORE], BF16, tag="hT")
            cT = pp.tile([128, HK * BCORE], F32, tag="cT")
            ctxT = pp.tile([128, HK * BCORE], BF16, tag="ctxT")
            nc.vector.memset(hT[:], 0.0)
            nc.vector.memset(cT[:], 0.0)

            enc_sb, hproj, ad = [], [], []
            for c in range(nchunk):
                e_ = pp.tile([128, nbt * 512], BF16, tag=f"enc{c}")
                for q in range(4):
                    w = nbt * 512 // 4
                    nc.sync.dma_start(
                        e_[:, q * w : (q + 1) * w], d_enc[c, :, q * w : (q + 1) * w]
                    )
                enc_sb.append(e_)
                hproj.append(
                    pp.tile([128, HK * bt], BF16, tag=f"hproj{c}", name=f"hproj{c}")
                )
                a_ = pp.tile([128, bc], BF16, tag=f"ad{c}", name=f"ad{c}")
                nc.vector.memset(a_[:], 0.0)
                ad.append(a_)

            # ---- init: H_projT = w_i2h @ encT + b_h2h ----
            # encT pool is scoped: its recycled addresses give later writers
            # WAW deps on the input DMA queues, but bacc's event-semaphore
            # pass legalizes the wait fan-in.
            with (
                tc.tile_pool(name="encT", bufs=1) as pet,
                tc.tile_pool(name="initps", bufs=4, space="PSUM") as pips,
            ):
                for c in range(nchunk):
                    et = pet.tile([128, HK * bt], BF16, tag=f"encT{c}", name=f"encT{c}")
                    for q in range(4):
                        w = HK * bt // 4
                        nc.sync.dma_start(
                            et[:, q * w : (q + 1) * w],
                            d_encT[c, :, q * w : (q + 1) * w],
                        )
                    for m in range(HK):
                        for n in range(bt // 512):
                            ps = pips.tile([128, 512], F32, tag="initp")
                            for k in range(HK):
                                nc.tensor.matmul(
                                    ps[:],
                                    w_i2h[:, k * H + 128 * m : k * H + 128 * m + 128],
                                    et[:, k * bt + 512 * n : k * bt + 512 * n + 512],
                                    start=(k == 0),
                                    stop=(k == HK - 1),
                                )
                            nc.scalar.activation(
                                hproj[c][:, m * bt + 512 * n : m * bt + 512 * n + 512],
                                ps[:],
                                AF.Identity,
                                bias=b_h2h[:, m : m + 1],
                            )

            # ---- decode steps ----
            with (
                tc.tile_pool(name="work", bufs=6) as pw,
                tc.tile_pool(name="small", bufs=4) as psm,
                tc.tile_pool(name="ps_hp", bufs=1, space="PSUM") as ps_hp,
                tc.tile_pool(name="ps_e", bufs=2, space="PSUM") as ps_e,
                tc.tile_pool(name="ps_tr", bufs=2, space="PSUM") as ps_tr,
                tc.tile_pool(name="ps_ctx", bufs=1, space="PSUM") as ps_ctx,
                tc.tile_pool(name="ps_g", bufs=2, space="PSUM") as ps_g,
            ):
                for s in range(steps):
                    step_body(
                        nc, s, steps, nchunk, bc, bt,
                        pw, psm, ps_hp, ps_e, ps_tr, ps_ctx, ps_g,
                        enc_sb, hproj, hT, cT, ctxT, ad,
                        w_h2h, w_sc, w_ctx, w_hh, w_oh, w_gen, b_gen,
                        ohT, ones, id_f, d_out,
                    )
    if not nc.is_finalized():
        nc.finalize()
    return nc


def step_body(
    nc, s, steps, nchunk, bc, bt,
    pw, psm, ps_hp, ps_e, ps_tr, ps_ctx, ps_g,
    enc_sb, hproj, hT, cT, ctxT, ad,
    w_h2h, w_sc, w_ctx, w_hh, w_oh, w_gen, b_gen,
    ohT, ones, id_f, d_out,
):
    nj = bt // 128
    BW = nchunk * bc  # fused col-block width (BCORE)

    # -- snapshot h (overwritten per-k during LSTM update below) --
    h_use = psm.tile([128, HK * BW], BF16, tag="h_use")
    nc.vector.tensor_copy(h_use[:], hT[:])

    # -- hp = w_h2h @ h (both chunks fused, N=BW) --
    php = ps_hp.tile([128, HK * BW], F32, tag="hp")
    for m in range(HK):
        for k in range(HK):
            nc.tensor.matmul(
                php[:, m * BW : (m + 1) * BW],
                w_h2h[:, k * H + 128 * m : k * H + 128 * m + 128],
                h_use[:, k * BW : (k + 1) * BW],
                start=(k == 0),
                stop=(k == HK - 1),
            )

    # -- attention per chunk (these pipeline against each other) --
    for c in range(nchunk):
        # duplicate-x2 hp copy (enables DVE 2x mode on the broadcast add)
        hp2 = psm.tile([128, HK * bc * 2], BF16, tag=f"hp2_{c}")
        phv = php[:].rearrange("p (m b2) -> p m b2", m=HK)[:, :, c * bc : (c + 1) * bc]
        nc.vector.tensor_copy(
            hp2[:].rearrange("p (m b two) -> p m b two", m=HK, two=2),
            phv.unsqueeze(3).broadcast_to([128, HK, bc, 2]),
        )
        hp2v = hp2[:].rearrange("p (m b two) -> p m b two", m=HK, two=2)

        ths = []
        for k in range(HK):
            sl = hproj[c][:, k * bt : (k + 1) * bt]
            a = pw.tile([128, bt], BF16, tag=f"a{c}", bufs=2)
            nc.vector.tensor_add(
                a[:].rearrange("p (b t2 two) -> p b t2 two", b=bc, two=2),
                sl.rearrange("p (b t2 two) -> p b t2 two", b=bc, two=2),
                hp2v[:, k].unsqueeze(2).broadcast_to([128, bc, T // 2, 2]),
            )
            th = pw.tile([128, bt], BF16, tag=f"th{c}", bufs=4)
            nc.scalar.activation(th[:], a[:], AF.Tanh)
            ths.append(th)
        pe2 = ps_e.tile([128, nj], F32, tag="e2")
        for j in range(nj):
            for k in range(HK):
                nc.tensor.matmul(
                    pe2[:, j : j + 1],
                    ths[k][:, 128 * j : 128 * j + 128],
                    w_sc[:, k : k + 1],
                    start=(k == 0),
                    stop=(k == HK - 1),
                    skip_group_check=True,
                )
        e2 = psm.tile([128, nj], F32, tag="e2sb")
        nc.vector.tensor_copy(e2[:], pe2[:])

        # transpose e to [nj, (2b x t)], softmax over t (fp32; no max-sub:
        # |e| <= ||w_score||_1 ~ 20 so fp32 exp is safe)
        ptr = ps_tr.tile([nj, 128], F32, tag="tr")
        nc.tensor.transpose(ptr[:], e2[:], id_f[:])
        ex = psm.tile([nj, 128], F32, tag="ex")
        nc.scalar.activation(ex[:], ptr[:], AF.Exp)
        ssum = psm.tile([nj, 2], F32, tag="ssum")
        nc.vector.reduce_sum(
            ssum[:], ex[:].rearrange("p (b t) -> p b t", b=2), axis=mybir.AxisListType.X
        )
        rinv = psm.tile([nj, 2], F32, tag="rinv")
        nc.vector.reciprocal(rinv[:], ssum[:])
        al = psm.tile([nj, 128], F32, tag="al")
        nc.vector.tensor_mul(
            al[:].rearrange("p (b t) -> p b t", b=2),
            ex[:].rearrange("p (b t) -> p b t", b=2),
            rinv[:].unsqueeze(2).broadcast_to([nj, 2, T]),
        )

        # alpha back to bt-partitions; build block-diag lhsT bands
        pac = ps_tr.tile([128, nj], F32, tag="tr")
        nc.tensor.transpose(pac[:], al[:], id_f[0:nj, 0:nj])
        acol = psm.tile([128, nj], BF16, tag="acol")
        nc.vector.tensor_copy(acol[:], pac[:])
        adv = ad[c][:].rearrange("p (i two) -> p i two", two=2)
        for jj in range(2):
            nc.vector.tensor_copy(
                adv[64 * jj : 64 * jj + 64, :, jj], acol[64 * jj : 64 * jj + 64, :]
            )

        # ctxT[d, b] direct: lhsT = enc tile (d -> partitions), rhs = ad pair
        pctxT = ps_ctx.tile([128, HK * bc], F32, tag="ctxT_ps")
        for m in range(HK):
            for i in range(bc // 2):
                nc.tensor.matmul(
                    pctxT[:, m * bc + 2 * i : m * bc + 2 * i + 2],
                    enc_sb[c][:, 512 * i + 128 * m : 512 * i + 128 * m + 128],
                    ad[c][:, 2 * i : 2 * i + 2],
                    start=True,
                    stop=True,
                )
        nc.vector.tensor_copy(
            ctxT[:].rearrange("p (k b2) -> p k b2", k=HK)[:, :, c * bc : (c + 1) * bc],
            pctxT[:].rearrange("p (k b) -> p k b", k=HK),
        )

    # -- gates (fused chunks, cols [i | f | o | g] per h-tile k) + LSTM --
    ohsl = ohT[:, s * BW : (s + 1) * BW]
    for k in range(HK):
        pg = ps_g.tile([128, 4 * BW], F32, tag="g")
        for gi, gate in enumerate((0, 1, 3, 2)):  # cols = [i, f, o, g]
            m = 4 * gate + k
            col = pg[:, gi * BW : (gi + 1) * BW]
            for kk in range(HK):
                nc.tensor.matmul(
                    col,
                    w_ctx[:, kk * G4 + 128 * m : kk * G4 + 128 * m + 128],
                    ctxT[:, kk * BW : (kk + 1) * BW],
                    start=(kk == 0),
                    stop=False,
                    skip_group_check=True,
                )
            nc.tensor.matmul(
                col, w_oh[:, 128 * m : 128 * m + 128], ohsl,
                start=False, stop=False, skip_group_check=True,
            )
            for kk in range(HK):
                nc.tensor.matmul(
                    col,
                    w_hh[:, kk * G4 + 128 * m : kk * G4 + 128 * m + 128],
                    h_use[:, kk * BW : (kk + 1) * BW],
                    start=False,
                    stop=(kk == HK - 1),
                    skip_group_check=True,
                )
        tifo = psm.tile([128, 3 * BW], F32, tag="tifo")
        nc.scalar.activation(tifo[:], pg[:, 0 : 3 * BW], AF.Tanh, scale=0.5)
        tg = psm.tile([128, BW], F32, tag="tg")
        nc.scalar.activation(tg[:], pg[:, 3 * BW : 4 * BW], AF.Tanh)
        sifo = psm.tile([128, 3 * BW], F32, tag="sifo")
        nc.vector.tensor_scalar(sifo[:], tifo[:], 0.5, 0.5, ALU.mult, ALU.add)
        m1 = psm.tile([128, BW], F32, tag="m1")
        nc.vector.tensor_mul(m1[:], sifo[:, BW : 2 * BW], cT[:, k * BW : (k + 1) * BW])
        m2 = psm.tile([128, BW], F32, tag="m2")
        nc.vector.tensor_mul(m2[:], sifo[:, 0:BW], tg[:])
        nc.vector.tensor_add(cT[:, k * BW : (k + 1) * BW], m1[:], m2[:])
        tc_ = psm.tile([128, BW], F32, tag="tc")
        nc.scalar.activation(tc_[:], cT[:, k * BW : (k + 1) * BW], AF.Tanh)
        nc.vector.tensor_mul(
            hT[:, k * BW : (k + 1) * BW], sifo[:, 2 * BW : 3 * BW], tc_[:]
        )

    # -- probs = hT.T @ w_genT + b_gen -> DRAM --
    pp_ = ps_e.tile([BW, C], F32, tag="e2")
    for k in range(HK):
        nc.tensor.matmul(
            pp_[:],
            hT[:, k * BW : (k + 1) * BW],
            w_gen[:, k * C : (k + 1) * C],
            start=(k == 0),
            stop=False,
            skip_group_check=True,
        )
    nc.tensor.matmul(
        pp_[:], ones[0:1, 0:BW], b_gen[:], start=False, stop=True, skip_group_check=True
    )
    po = psm.tile([BW, C], F32, tag="po")
    nc.vector.tensor_copy(po[:], pp_[:])
    nc.sync.dma_start(d_out[:, s, :], po[:])


# ------------------------- host side -------------------------


def prep_inputs(encoder_output, text, w_i2h, w_h2h, b_h2h, w_score, w_ih, w_hh,
                b_ih, b_hh, w_gen, b_gen, steps=S, nchunk=NCHUNK):
    """Build per-core input maps (numpy only)."""
    bc = BCORE // nchunk
    bt = bc * T
    enc = np.asarray(encoder_output, np.float32)
    text = np.asarray(text)

    wid = {}
    wid["w_i2hT"] = _tile128(np.asarray(w_i2h, np.float32).T.astype(BF))
    wid["w_h2hT"] = _tile128(np.asarray(w_h2h, np.float32).T.astype(BF))
    wid["w_scoreT"] = _tile128(np.asarray(w_score, np.float32).reshape(H, 1).astype(BF))
    wid["w_ctxT"] = _tile128(np.asarray(w_ih, np.float32)[:, :D].T.astype(BF))
    wid["w_hhT"] = _tile128(np.asarray(w_hh, np.float32).T.astype(BF))
    woh = np.zeros((128, G4), BF)  # K padded to 128 so FWL kicks in
    woh[:C] = np.asarray(w_ih, np.float32)[:, D:].T.astype(BF)
    woh[C] = (np.asarray(b_ih, np.float32) + np.asarray(b_hh, np.float32)).astype(BF)
    wid["w_ohT"] = woh
    wid["w_genT"] = _tile128(np.asarray(w_gen, np.float32).T.astype(BF))
    wid["b_gen"] = np.asarray(b_gen, np.float32).reshape(1, C).astype(BF)
    wid["b_h2hT"] = np.ascontiguousarray(
        np.asarray(b_h2h, np.float32).reshape(HK, 128).T
    )
    wid["id_f32"] = np.eye(128, dtype=np.float32)
    wid["ones_row"] = np.ones((1, BCORE), BF)

    in_maps = []
    for core in range(NCORES):
        rows = slice(core * BCORE, (core + 1) * BCORE)
        ec = enc[rows]  # [64, T, D]
        enc_sb = np.zeros((nchunk, 128, (bt // 128) * 512), BF)
        encT_sb = np.zeros((nchunk, 128, HK * bt), BF)
        for c in range(nchunk):
            flat = ec[c * bc : (c + 1) * bc].reshape(bt, D)  # b-major (b,t) rows
            enc_sb[c] = _tile128(flat.astype(BF))
            encT_sb[c] = _tile128(np.ascontiguousarray(flat.T).astype(BF))
        oh = np.zeros((128, steps * BCORE), BF)
        tx = text[rows]  # [64, S]
        for s in range(steps):
            oh[tx[:, s].astype(np.int64), s * BCORE + np.arange(BCORE)] = 1.0
        oh[C] = 1.0
        m = dict(wid)
        m["enc_sb"] = enc_sb
        m["encT_sb"] = encT_sb
        m["ohT_sb"] = oh
        in_maps.append(m)
    return in_maps


_NC_CACHE = {}


def get_nc(steps=S, nchunk=NCHUNK):
    key = (steps, nchunk)
    if key not in _NC_CACHE:
        _NC_CACHE[key] = build_nc(steps, nchunk)
    return _NC_CACHE[key]


def run(inputs, steps=S, nchunk=NCHUNK, trace=False):
    nc = get_nc(steps, nchunk)
    in_maps = prep_inputs(**inputs, steps=steps, nchunk=nchunk)
    res = run_bass_kernel_spmd(nc, in_maps, list(range(NCORES)), trace=trace)
    out = np.concatenate([res.results[i]["probs"] for i in range(NCORES)], axis=0)
    return out.astype(np.float32), res


def kernel(**inputs):
    out, _ = run(inputs)
    return out


# revision 16
# speedup vs baseline: 1.3510x; 1.2629x over previous
"""Trainium2 Bass kernel: attention-LSTM decoder (nn_Attention_74698071212133).

Sharding: data-parallel over batch across 8 NeuronCores (64 rows each), weights
replicated.  Each core splits its 64 rows into NCHUNK chunks whose attention
phases pipeline against each other; the LSTM/gates phase is fused across chunks
(wider matmuls, N=64) since the recurrence joins there anyway.

Per-core, per step s (batch b=32/chunk, T=64, H=512, C=38):
  h_use = snapshot(hT)                         (DVE)
  hpT   = w_h2h @ h_use       [fused chunks]   (PE)
  per chunk: a = H_projT + bcast_t(hpT)        (DVE, bf16 2x, 2048-wide)
             th = tanh(a)                      (ACT)
             e  = w_score . th -> psum[bt,16]  (PE, lhsT=th slices)
             softmax via PE transpose -> alpha (PE/DVE/ACT, fp32)
             ctxT = enc.T @ alpha-blockdiag    (PE, direct [d, b] layout)
  gatesT = W[ctxT; oh; 1; h_use] [fused]       (PE, K=128-padded onehots)
  lstm elementwise, sigmoid via 0.5*tanh(x/2)+0.5  (ACT+DVE)
  probs = hT.T @ w_genT + b_gen -> DRAM        (PE + DMA)
"""

import sys

sys.path.insert(0, "/opt/trn_rl_repo")

import numpy as np
import ml_dtypes

import concourse.bass as bass
import concourse.mybir as mybir
import concourse.tile as tile
from concourse import bacc
from concourse.bass_utils import run_bass_kernel_spmd

BF = ml_dtypes.bfloat16
F32 = mybir.dt.float32
BF16 = mybir.dt.bfloat16
AF = mybir.ActivationFunctionType
ALU = mybir.AluOpType

# Problem constants
B, T, D, H, C, S = 512, 64, 512, 512, 38, 26
NCORES = 8
BCORE = B // NCORES  # 64
NCHUNK = 2
G4 = 4 * H  # 2048
HK = H // 128  # 4 h-tiles


def _tile128(a):
    """[R, N] with R = r*128 -> [128, r*N] col-block layout (block k = rows 128k..)."""
    r = a.shape[0] // 128
    return np.ascontiguousarray(
        a.reshape(r, 128, a.shape[1]).transpose(1, 0, 2).reshape(128, -1)
    )


def build_nc(steps=S, nchunk=NCHUNK):
    bc = BCORE // nchunk  # batch per chunk
    bt = bc * T  # flattened (b, t) per chunk, b-major
    nbt = bt // 128  # 128-row bt tiles per chunk

    nc = bacc.Bacc()
    dp = nc.declare_dram_parameter
    # Per-core tensors (pre-tiled on host into [128, cols] SBUF images)
    d_enc = dp("enc_sb", [nchunk, 128, nbt * 512], BF16, isOutput=False)
    d_encT = dp("encT_sb", [nchunk, 128, HK * bt], BF16, isOutput=False)
    d_oh = dp("ohT_sb", [128, steps * BCORE], BF16, isOutput=False)
    # Replicated weights
    d_wi2h = dp("w_i2hT", [128, HK * H], BF16, isOutput=False)
    d_wh2h = dp("w_h2hT", [128, HK * H], BF16, isOutput=False)
    d_wsc = dp("w_scoreT", [128, HK], BF16, isOutput=False)
    d_wctx = dp("w_ctxT", [128, HK * G4], BF16, isOutput=False)
    d_whh = dp("w_hhT", [128, HK * G4], BF16, isOutput=False)
    d_woh = dp("w_ohT", [128, G4], BF16, isOutput=False)
    d_wgen = dp("w_genT", [128, HK * C], BF16, isOutput=False)
    d_bgen = dp("b_gen", [1, C], BF16, isOutput=False)
    d_bh2h = dp("b_h2hT", [128, HK], F32, isOutput=False)
    d_idf = dp("id_f32", [128, 128], F32, isOutput=False)
    d_ones = dp("ones_row", [1, BCORE], BF16, isOutput=False)
    d_out = dp("probs", [BCORE, steps, C], F32, isOutput=True)

    with tile.TileContext(nc) as tc:
        with (
            tc.tile_pool(name="consts", bufs=1) as pc,
            tc.tile_pool(name="persist", bufs=1) as pp,
        ):
            # ---- load constants ----
            def cload(dram, shape, dt):
                t_ = pc.tile(list(shape), dt, name=dram.tensor.name + "_sb")
                nc.sync.dma_start(t_[:], dram)
                return t_

            w_i2h = cload(d_wi2h[:], [128, HK * H], BF16)
            w_h2h = cload(d_wh2h[:], [128, HK * H], BF16)
            w_sc = cload(d_wsc[:], [128, HK], BF16)
            w_ctx = cload(d_wctx[:], [128, HK * G4], BF16)
            w_hh = cload(d_whh[:], [128, HK * G4], BF16)
            w_oh = cload(d_woh[:], [128, G4], BF16)
            w_gen = cload(d_wgen[:], [128, HK * C], BF16)
            b_gen = cload(d_bgen[:], [1, C], BF16)
            b_h2h = cload(d_bh2h[:], [128, HK], F32)
            id_f = cload(d_idf[:], [128, 128], F32)
            ones = cload(d_ones[:], [1, BCORE], BF16)
            ohT = cload(d_oh[:], [128, steps * BCORE], BF16)

            # ---- persistent state (fused layout: col-block k is BCORE wide,
            #      [chunk0 bc | chunk1 bc]) ----
            hT = pp.tile([128, HK * BCORE], BF16, tag="hT")
            cT = pp.tile([128, HK * BCORE], F32, tag="cT")
            ctxT = pp.tile([128, HK * BCORE], BF16, tag="ctxT")
            nc.vector.memset(hT[:], 0.0)
            nc.vector.memset(cT[:], 0.0)

            enc_sb, hproj, ad = [], [], []
            for c in range(nchunk):
                e_ = pp.tile([128, nbt * 512], BF16, tag=f"enc{c}")
                for q in range(4):
                    w = nbt * 512 // 4
                    nc.sync.dma_start(
                        e_[:, q * w : (q + 1) * w], d_enc[c, :, q * w : (q + 1) * w]
                    )
                enc_sb.append(e_)
                hproj.append(
                    pp.tile([128, HK * bt], BF16, tag=f"hproj{c}", name=f"hproj{c}")
                )
                a_ = pp.tile([128, bc], BF16, tag=f"ad{c}", name=f"ad{c}")
                nc.vector.memset(a_[:], 0.0)
                ad.append(a_)

            # ---- init: H_projT = w_i2h @ encT + b_h2h ----
            # encT pool is scoped: its recycled addresses give later writers
            # WAW deps on the input DMA queues, but bacc's event-semaphore
            # pass legalizes the wait fan-in.
            with (
                tc.tile_pool(name="encT", bufs=1) as pet,
                tc.tile_pool(name="initps", bufs=4, space="PSUM") as pips,
            ):
                for c in range(nchunk):
                    et = pet.tile([128, HK * bt], BF16, tag=f"encT{c}", name=f"encT{c}")
                    for q in range(4):
                        w = HK * bt // 4
                        nc.sync.dma_start(
                            et[:, q * w : (q + 1) * w],
                            d_encT[c, :, q * w : (q + 1) * w],
                        )
                    for m in range(HK):
                        for n in range(bt // 512):
                            ps = pips.tile([128, 512], F32, tag="initp")
                            for k in range(HK):
                                nc.tensor.matmul(
                                    ps[:],
                                    w_i2h[:, k * H + 128 * m : k * H + 128 * m + 128],
                                    et[:, k * bt + 512 * n : k * bt + 512 * n + 512],
                                    start=(k == 0),
                                    stop=(k == HK - 1),
                                )
                            nc.scalar.activation(
                                hproj[c][:, m * bt + 512 * n : m * bt + 512 * n + 512],
                                ps[:],
                                AF.Identity,
                                bias=b_h2h[:, m : m + 1],
                            )

            # ---- decode steps ----
            with (
                tc.tile_pool(name="work", bufs=6) as pw,
                tc.tile_pool(name="small", bufs=4) as psm,
                tc.tile_pool(name="ps_hp", bufs=1, space="PSUM") as ps_hp,
                tc.tile_pool(name="ps_e", bufs=2, space="PSUM") as ps_e,
                tc.tile_pool(name="ps_tr", bufs=1, space="PSUM") as ps_tr,
                tc.tile_pool(name="ps_ctx", bufs=1, space="PSUM") as ps_ctx,
                tc.tile_pool(name="ps_g", bufs=2, space="PSUM") as ps_g,
            ):
                php_holder = [None]
                for s in range(steps):
                    step_body(
                        nc, s, steps, nchunk, bc, bt,
                        pw, psm, ps_hp, ps_e, ps_tr, ps_ctx, ps_g,
                        enc_sb, hproj, hT, cT, ctxT, ad,
                        w_h2h, w_sc, w_ctx, w_hh, w_oh, w_gen, b_gen,
                        ohT, ones, id_f, d_out, php_holder,
                    )
    if not nc.is_finalized():
        nc.finalize()
    return nc


def step_body(
    nc, s, steps, nchunk, bc, bt,
    pw, psm, ps_hp, ps_e, ps_tr, ps_ctx, ps_g,
    enc_sb, hproj, hT, cT, ctxT, ad,
    w_h2h, w_sc, w_ctx, w_hh, w_oh, w_gen, b_gen,
    ohT, ones, id_f, d_out, php_holder,
):
    nj = bt // 128
    BW = nchunk * bc  # fused col-block width (BCORE)
    php = php_holder[0]  # hp psum computed during the previous step's LSTM

    # -- snapshot h (overwritten per-k during LSTM update below) --
    h_use = psm.tile([128, HK * BW], BF16, tag="h_use")
    nc.vector.tensor_copy(h_use[:], hT[:])

    # -- attention per chunk (these pipeline against each other) --
    if s > 0:
        hps = psm.tile([128, HK * BW], F32, tag="hps")
        nc.vector.tensor_copy(hps[:], php[:, 0 : HK * BW])
        for kk in range(1, HK):
            nc.vector.tensor_add(
                hps[:], hps[:], php[:, kk * HK * BW : (kk + 1) * HK * BW]
            )
    for c in range(nchunk):
        if s > 0:
            # duplicate-x2 hp copy (enables DVE 2x mode on the broadcast add)
            hp2 = psm.tile([128, HK * bc * 2], BF16, tag=f"hp2_{c}")
            phv = hps[:].rearrange("p (m b2) -> p m b2", m=HK)[
                :, :, c * bc : (c + 1) * bc
            ]
            nc.vector.tensor_copy(
                hp2[:].rearrange("p (m b two) -> p m b two", m=HK, two=2),
                phv.unsqueeze(3).broadcast_to([128, HK, bc, 2]),
            )
            hp2v = hp2[:].rearrange("p (m b two) -> p m b two", m=HK, two=2)

        # e scores: per-k matmuls into separate psum blocks (no accumulation
        # groups -> each runs right after its tanh), DVE tree-sum at the end
        pe2 = ps_e.tile([128, HK * nj], F32, tag="e2")
        for k in range(HK):
            sl = hproj[c][:, k * bt : (k + 1) * bt]
            if s == 0:
                th = pw.tile([128, bt], BF16, tag=f"th{c}", bufs=4)
                nc.scalar.activation(th[:], sl, AF.Tanh)
            else:
                a = pw.tile([128, bt], BF16, tag=f"a{c}", bufs=2)
                nc.vector.tensor_add(
                    a[:].rearrange("p (b t2 two) -> p b t2 two", b=bc, two=2),
                    sl.rearrange("p (b t2 two) -> p b t2 two", b=bc, two=2),
                    hp2v[:, k].unsqueeze(2).broadcast_to([128, bc, T // 2, 2]),
                )
                th = pw.tile([128, bt], BF16, tag=f"th{c}", bufs=4)
                nc.scalar.activation(th[:], a[:], AF.Tanh)
            for j in range(nj):
                nc.tensor.matmul(
                    pe2[:, k * nj + j : k * nj + j + 1],
                    th[:, 128 * j : 128 * j + 128],
                    w_sc[:, k : k + 1],
                    start=True,
                    stop=True,
                )
        e2 = psm.tile([128, nj], F32, tag="e2sb")
        nc.vector.tensor_copy(e2[:], pe2[:, 0:nj])
        for kk in range(1, HK):
            nc.vector.tensor_add(e2[:], e2[:], pe2[:, kk * nj : (kk + 1) * nj])

        # transpose e to [nj, (2b x t)], softmax over t (fp32; no max-sub:
        # |e| <= ||w_score||_1 ~ 20 so fp32 exp is safe)
        ptr = ps_tr.tile([nj, 128], F32, tag="tr")
        nc.tensor.transpose(ptr[:], e2[:], id_f[:])
        ex = psm.tile([nj, 128], F32, tag="ex")
        nc.scalar.activation(ex[:], ptr[:], AF.Exp)
        ssum = psm.tile([nj, 2], F32, tag="ssum")
        nc.vector.reduce_sum(
            ssum[:], ex[:].rearrange("p (b t) -> p b t", b=2), axis=mybir.AxisListType.X
        )
        rinv = psm.tile([nj, 2], F32, tag="rinv")
        nc.vector.reciprocal(rinv[:], ssum[:])
        al = psm.tile([nj, 128], F32, tag="al")
        nc.vector.tensor_mul(
            al[:].rearrange("p (b t) -> p b t", b=2),
            ex[:].rearrange("p (b t) -> p b t", b=2),
            rinv[:].unsqueeze(2).broadcast_to([nj, 2, T]),
        )

        # alpha back to bt-partitions; build block-diag lhsT bands
        pac = ps_tr.tile([128, nj], F32, tag="tr")
        nc.tensor.transpose(pac[:], al[:], id_f[0:nj, 0:nj])
        adv = ad[c][:].rearrange("p (i two) -> p i two", two=2)
        for jj in range(2):
            nc.vector.tensor_copy(
                adv[64 * jj : 64 * jj + 64, :, jj], pac[64 * jj : 64 * jj + 64, :]
            )

        # ctxT[d, b] direct: lhsT = enc tile (d -> partitions), rhs = ad pair
        pctxT = ps_ctx.tile([128, HK * bc], F32, tag="ctxT_ps")
        for m in range(HK):
            for i in range(bc // 2):
                nc.tensor.matmul(
                    pctxT[:, m * bc + 2 * i : m * bc + 2 * i + 2],
                    enc_sb[c][:, 512 * i + 128 * m : 512 * i + 128 * m + 128],
                    ad[c][:, 2 * i : 2 * i + 2],
                    start=True,
                    stop=True,
                )
        nc.vector.tensor_copy(
            ctxT[:].rearrange("p (k b2) -> p k b2", k=HK)[:, :, c * bc : (c + 1) * bc],
            pctxT[:].rearrange("p (k b) -> p k b", k=HK),
        )

    # -- gates (fused chunks, cols [i | f | o | g] per h-tile k) + LSTM --
    ohsl = ohT[:, s * BW : (s + 1) * BW]
    for k in range(HK):
        pg = ps_g.tile([128, 4 * BW], F32, tag="g")
        for gi, gate in enumerate((0, 1, 3, 2)):  # cols = [i, f, o, g]
            m = 4 * gate + k
            col = pg[:, gi * BW : (gi + 1) * BW]
            for kk in range(HK):
                nc.tensor.matmul(
                    col,
                    w_ctx[:, kk * G4 + 128 * m : kk * G4 + 128 * m + 128],
                    ctxT[:, kk * BW : (kk + 1) * BW],
                    start=(kk == 0),
                    stop=False,
                    skip_group_check=True,
                )
            nc.tensor.matmul(
                col, w_oh[:, 128 * m : 128 * m + 128], ohsl,
                start=False, stop=False, skip_group_check=True,
            )
            for kk in range(HK):
                nc.tensor.matmul(
                    col,
                    w_hh[:, kk * G4 + 128 * m : kk * G4 + 128 * m + 128],
                    h_use[:, kk * BW : (kk + 1) * BW],
                    start=False,
                    stop=(kk == HK - 1),
                    skip_group_check=True,
                )
        tifo = psm.tile([128, 3 * BW], F32, tag="tifo")
        nc.scalar.activation(tifo[:], pg[:, 0 : 3 * BW], AF.Tanh, scale=0.5)
        tg = psm.tile([128, BW], F32, tag="tg")
        nc.scalar.activation(tg[:], pg[:, 3 * BW : 4 * BW], AF.Tanh)
        sifo = psm.tile([128, 3 * BW], F32, tag="sifo")
        nc.vector.tensor_scalar(sifo[:], tifo[:], 0.5, 0.5, ALU.mult, ALU.add)
        m1 = psm.tile([128, BW], F32, tag="m1")
        nc.vector.tensor_mul(m1[:], sifo[:, BW : 2 * BW], cT[:, k * BW : (k + 1) * BW])
        m2 = psm.tile([128, BW], F32, tag="m2")
        nc.vector.tensor_mul(m2[:], sifo[:, 0:BW], tg[:])
        nc.vector.tensor_add(cT[:, k * BW : (k + 1) * BW], m1[:], m2[:])
        tc_ = psm.tile([128, BW], F32, tag="tc")
        nc.scalar.activation(tc_[:], cT[:, k * BW : (k + 1) * BW], AF.Tanh)
        nc.vector.tensor_mul(
            hT[:, k * BW : (k + 1) * BW], sifo[:, 2 * BW : 3 * BW], tc_[:]
        )
        if s < steps - 1:
            if k == 0:
                php_holder[0] = ps_hp.tile(
                    [128, HK * HK * BW], F32, tag="hp", name="php"
                )
            for m in range(HK):
                nc.tensor.matmul(
                    php_holder[0][
                        :, k * HK * BW + m * BW : k * HK * BW + (m + 1) * BW
                    ],
                    w_h2h[:, k * H + 128 * m : k * H + 128 * m + 128],
                    hT[:, k * BW : (k + 1) * BW],
                    start=True,
                    stop=True,
                )

    # -- probs = hT.T @ w_genT + b_gen -> DRAM --
    pp_ = ps_e.tile([BW, C], F32, tag="e2")
    for k in range(HK):
        nc.tensor.matmul(
            pp_[:],
            hT[:, k * BW : (k + 1) * BW],
            w_gen[:, k * C : (k + 1) * C],
            start=(k == 0),
            stop=False,
            skip_group_check=True,
        )
    nc.tensor.matmul(
        pp_[:], ones[0:1, 0:BW], b_gen[:], start=False, stop=True, skip_group_check=True
    )
    po = psm.tile([BW, C], F32, tag="po")
    nc.vector.tensor_copy(po[:], pp_[:])
    nc.sync.dma_start(d_out[:, s, :], po[:])


# ------------------------- host side -------------------------


def prep_inputs(encoder_output, text, w_i2h, w_h2h, b_h2h, w_score, w_ih, w_hh,
                b_ih, b_hh, w_gen, b_gen, steps=S, nchunk=NCHUNK):
    """Build per-core input maps (numpy only)."""
    bc = BCORE // nchunk
    bt = bc * T
    enc = np.asarray(encoder_output, np.float32)
    text = np.asarray(text)

    wid = {}
    wid["w_i2hT"] = _tile128(np.asarray(w_i2h, np.float32).T.astype(BF))
    wid["w_h2hT"] = _tile128(np.asarray(w_h2h, np.float32).T.astype(BF))
    wid["w_scoreT"] = _tile128(np.asarray(w_score, np.float32).reshape(H, 1).astype(BF))
    wid["w_ctxT"] = _tile128(np.asarray(w_ih, np.float32)[:, :D].T.astype(BF))
    wid["w_hhT"] = _tile128(np.asarray(w_hh, np.float32).T.astype(BF))
    woh = np.zeros((128, G4), BF)  # K padded to 128 so FWL kicks in
    woh[:C] = np.asarray(w_ih, np.float32)[:, D:].T.astype(BF)
    woh[C] = (np.asarray(b_ih, np.float32) + np.asarray(b_hh, np.float32)).astype(BF)
    wid["w_ohT"] = woh
    wid["w_genT"] = _tile128(np.asarray(w_gen, np.float32).T.astype(BF))
    wid["b_gen"] = np.asarray(b_gen, np.float32).reshape(1, C).astype(BF)
    wid["b_h2hT"] = np.ascontiguousarray(
        np.asarray(b_h2h, np.float32).reshape(HK, 128).T
    )
    wid["id_f32"] = np.eye(128, dtype=np.float32)
    wid["ones_row"] = np.ones((1, BCORE), BF)

    in_maps = []
    for core in range(NCORES):
        rows = slice(core * BCORE, (core + 1) * BCORE)
        ec = enc[rows]  # [64, T, D]
        enc_sb = np.zeros((nchunk, 128, (bt // 128) * 512), BF)
        encT_sb = np.zeros((nchunk, 128, HK * bt), BF)
        for c in range(nchunk):
            flat = ec[c * bc : (c + 1) * bc].reshape(bt, D)  # b-major (b,t) rows
            enc_sb[c] = _tile128(flat.astype(BF))
            encT_sb[c] = _tile128(np.ascontiguousarray(flat.T).astype(BF))
        oh = np.zeros((128, steps * BCORE), BF)
        tx = text[rows]  # [64, S]
        for s in range(steps):
            oh[tx[:, s].astype(np.int64), s * BCORE + np.arange(BCORE)] = 1.0
        oh[C] = 1.0
        m = dict(wid)
        m["enc_sb"] = enc_sb
        m["encT_sb"] = encT_sb
        m["ohT_sb"] = oh
        in_maps.append(m)
    return in_maps


_NC_CACHE = {}


def get_nc(steps=S, nchunk=NCHUNK):
    key = (steps, nchunk)
    if key not in _NC_CACHE:
        _NC_CACHE[key] = build_nc(steps, nchunk)
    return _NC_CACHE[key]


def run(inputs, steps=S, nchunk=NCHUNK, trace=False):
    nc = get_nc(steps, nchunk)
    in_maps = prep_inputs(**inputs, steps=steps, nchunk=nchunk)
    res = run_bass_kernel_spmd(nc, in_maps, list(range(NCORES)), trace=trace)
    out = np.concatenate([res.results[i]["probs"] for i in range(NCORES)], axis=0)
    return out.astype(np.float32), res


def kernel(**inputs):
    out, _ = run(inputs)
    return out


# revision 18
# speedup vs baseline: 1.4348x; 1.0620x over previous
"""Trainium2 Bass kernel: attention-LSTM decoder (nn_Attention_74698071212133).

Sharding: data-parallel over batch across 8 NeuronCores (64 rows each), weights
replicated.  Each core splits its 64 rows into NCHUNK chunks whose attention
phases pipeline against each other; the LSTM/gates phase is fused across chunks
(wider matmuls, N=64) since the recurrence joins there anyway.

Per-core, per step s (batch b=32/chunk, T=64, H=512, C=38):
  h_use = snapshot(hT)                         (DVE)
  hpT   = w_h2h @ h_use       [fused chunks]   (PE)
  per chunk: a = H_projT + bcast_t(hpT)        (DVE, bf16 2x, 2048-wide)
             th = tanh(a)                      (ACT)
             e  = w_score . th -> psum[bt,16]  (PE, lhsT=th slices)
             softmax via PE transpose -> alpha (PE/DVE/ACT, fp32)
             ctxT = enc.T @ alpha-blockdiag    (PE, direct [d, b] layout)
  gatesT = W[ctxT; oh; 1; h_use] [fused]       (PE, K=128-padded onehots)
  lstm elementwise, sigmoid via 0.5*tanh(x/2)+0.5  (ACT+DVE)
  probs = hT.T @ w_genT + b_gen -> DRAM        (PE + DMA)
"""

import sys

sys.path.insert(0, "/opt/trn_rl_repo")

import numpy as np
import ml_dtypes

import concourse.bass as bass
import concourse.mybir as mybir
import concourse.tile as tile
from concourse import bacc
from concourse.bass_utils import run_bass_kernel_spmd

BF = ml_dtypes.bfloat16
F32 = mybir.dt.float32
BF16 = mybir.dt.bfloat16
AF = mybir.ActivationFunctionType
ALU = mybir.AluOpType

# Problem constants
B, T, D, H, C, S = 512, 64, 512, 512, 38, 26
NCORES = 8
BCORE = B // NCORES  # 64
NCHUNK = 2
G4 = 4 * H  # 2048
HK = H // 128  # 4 h-tiles


def _tile128(a):
    """[R, N] with R = r*128 -> [128, r*N] col-block layout (block k = rows 128k..)."""
    r = a.shape[0] // 128
    return np.ascontiguousarray(
        a.reshape(r, 128, a.shape[1]).transpose(1, 0, 2).reshape(128, -1)
    )


def build_nc(steps=S, nchunk=NCHUNK):
    bc = BCORE // nchunk  # batch per chunk
    bt = bc * T  # flattened (b, t) per chunk, b-major
    nbt = bt // 128  # 128-row bt tiles per chunk

    nc = bacc.Bacc()
    dp = nc.declare_dram_parameter
    # Per-core tensors (pre-tiled on host into [128, cols] SBUF images)
    d_enc = dp("enc_sb", [nchunk, 128, nbt * 512], BF16, isOutput=False)
    d_encT = dp("encT_sb", [nchunk, 128, HK * bt], BF16, isOutput=False)
    d_oh = dp("ohT_sb", [128, steps * BCORE], BF16, isOutput=False)
    # Replicated weights
    d_wi2h = dp("w_i2hT", [128, HK * H], BF16, isOutput=False)
    d_wh2h = dp("w_h2hT", [128, HK * H], BF16, isOutput=False)
    d_wsc = dp("w_scoreT", [128, HK], BF16, isOutput=False)
    d_wctx = dp("w_ctxT", [128, HK * G4], BF16, isOutput=False)
    d_whh = dp("w_hhT", [128, HK * G4], BF16, isOutput=False)
    d_woh = dp("w_ohT", [128, G4], BF16, isOutput=False)
    d_wgen = dp("w_genT", [128, HK * C], BF16, isOutput=False)
    d_bgen = dp("b_gen", [1, C], BF16, isOutput=False)
    d_bh2h = dp("b_h2hT", [128, HK], F32, isOutput=False)
    d_idf = dp("id_f32", [128, 128], F32, isOutput=False)
    d_ones = dp("ones_row", [1, BCORE], BF16, isOutput=False)
    d_out = dp("probs", [BCORE, steps, C], F32, isOutput=True)

    with tile.TileContext(nc) as tc:
        with (
            tc.tile_pool(name="consts", bufs=1) as pc,
            tc.tile_pool(name="persist", bufs=1) as pp,
        ):
            # ---- load constants ----
            def cload(dram, shape, dt):
                t_ = pc.tile(list(shape), dt, name=dram.tensor.name + "_sb")
                nc.sync.dma_start(t_[:], dram)
                return t_

            w_i2h = cload(d_wi2h[:], [128, HK * H], BF16)
            w_h2h = cload(d_wh2h[:], [128, HK * H], BF16)
            w_sc = cload(d_wsc[:], [128, HK], BF16)
            w_ctx = cload(d_wctx[:], [128, HK * G4], BF16)
            w_hh = cload(d_whh[:], [128, HK * G4], BF16)
            w_oh = cload(d_woh[:], [128, G4], BF16)
            w_gen = cload(d_wgen[:], [128, HK * C], BF16)
            b_gen = cload(d_bgen[:], [1, C], BF16)
            b_h2h = cload(d_bh2h[:], [128, HK], F32)
            id_f = cload(d_idf[:], [128, 128], F32)
            ones = cload(d_ones[:], [1, BCORE], BF16)
            ohT = cload(d_oh[:], [128, steps * BCORE], BF16)

            # ---- persistent state (fused layout: col-block k is BCORE wide,
            #      [chunk0 bc | chunk1 bc]) ----
            hT = pp.tile([128, HK * BCORE], BF16, tag="hT")
            cT = pp.tile([128, HK * BCORE], F32, tag="cT")
            ctxT = pp.tile([128, HK * BCORE], BF16, tag="ctxT")
            nc.vector.memset(hT[:], 0.0)
            nc.vector.memset(cT[:], 0.0)

            enc_sb, hproj, ad = [], [], []
            for c in range(nchunk):
                e_ = pp.tile([128, nbt * 512], BF16, tag=f"enc{c}")
                for q in range(4):
                    w = nbt * 512 // 4
                    nc.sync.dma_start(
                        e_[:, q * w : (q + 1) * w], d_enc[c, :, q * w : (q + 1) * w]
                    )
                enc_sb.append(e_)
                hproj.append(
                    pp.tile([128, HK * bt], BF16, tag=f"hproj{c}", name=f"hproj{c}")
                )
                a_ = pp.tile([128, bc], BF16, tag=f"ad{c}", name=f"ad{c}")
                nc.vector.memset(a_[:], 0.0)
                ad.append(a_)

            # ---- init: H_projT = w_i2h @ encT + b_h2h ----
            # encT pool is scoped: its recycled addresses give later writers
            # WAW deps on the input DMA queues, but bacc's event-semaphore
            # pass legalizes the wait fan-in.
            with (
                tc.tile_pool(name="encT", bufs=1) as pet,
                tc.tile_pool(name="initps", bufs=4, space="PSUM") as pips,
            ):
                for c in range(nchunk):
                    et = pet.tile([128, HK * bt], BF16, tag=f"encT{c}", name=f"encT{c}")
                    for q in range(4):
                        w = HK * bt // 4
                        nc.sync.dma_start(
                            et[:, q * w : (q + 1) * w],
                            d_encT[c, :, q * w : (q + 1) * w],
                        )
                    for m in range(HK):
                        for n in range(bt // 512):
                            ps = pips.tile([128, 512], F32, tag="initp")
                            for k in range(HK):
                                nc.tensor.matmul(
                                    ps[:],
                                    w_i2h[:, k * H + 128 * m : k * H + 128 * m + 128],
                                    et[:, k * bt + 512 * n : k * bt + 512 * n + 512],
                                    start=(k == 0),
                                    stop=(k == HK - 1),
                                )
                            nc.scalar.activation(
                                hproj[c][:, m * bt + 512 * n : m * bt + 512 * n + 512],
                                ps[:],
                                AF.Identity,
                                bias=b_h2h[:, m : m + 1],
                            )

            # ---- decode steps ----
            with (
                tc.tile_pool(name="work", bufs=6) as pw,
                tc.tile_pool(name="small", bufs=4) as psm,
                tc.tile_pool(name="ps_mix", bufs=2, space="PSUM") as ps_mix,
                tc.tile_pool(name="ps_tr", bufs=1, space="PSUM") as ps_tr,
                tc.tile_pool(name="ps_ctx", bufs=1, space="PSUM") as ps_ctx,
                tc.tile_pool(name="ps_g", bufs=4, space="PSUM") as ps_g,
            ):
                php_holder = [None]
                for s in range(steps):
                    step_body(
                        nc, s, steps, nchunk, bc, bt,
                        pw, psm, ps_mix, ps_tr, ps_ctx, ps_g,
                        enc_sb, hproj, hT, cT, ctxT, ad,
                        w_h2h, w_sc, w_ctx, w_hh, w_oh, w_gen, b_gen,
                        ohT, ones, id_f, d_out, php_holder,
                    )
    if not nc.is_finalized():
        nc.finalize()
    return nc


def step_body(
    nc, s, steps, nchunk, bc, bt,
    pw, psm, ps_mix, ps_tr, ps_ctx, ps_g,
    enc_sb, hproj, hT, cT, ctxT, ad,
    w_h2h, w_sc, w_ctx, w_hh, w_oh, w_gen, b_gen,
    ohT, ones, id_f, d_out, php_holder,
):
    nj = bt // 128
    BW = nchunk * bc  # fused col-block width (BCORE)
    php = php_holder[0]  # hp psum computed during the previous step's LSTM

    # -- snapshot h (overwritten per-k during LSTM update below) --
    h_use = psm.tile([128, HK * BW], BF16, tag="h_use")
    nc.vector.tensor_copy(h_use[:], hT[:])

    ohsl = ohT[:, s * BW : (s + 1) * BW]

    # -- attention per chunk (these pipeline against each other) --
    if s > 0:
        hps = psm.tile([128, HK * BW], F32, tag="hps")
        nc.vector.reduce_sum(
            hps[:],
            php[:].rearrange("p (k x) -> p x k", k=2),
            axis=mybir.AxisListType.X,
        )
    for c in range(nchunk):
        if s > 0:
            # duplicate-x2 hp copy (enables DVE 2x mode on the broadcast add)
            hp2 = psm.tile([128, HK * bc * 2], BF16, tag=f"hp2_{c}")
            phv = hps[:].rearrange("p (m b2) -> p m b2", m=HK)[
                :, :, c * bc : (c + 1) * bc
            ]
            nc.vector.tensor_copy(
                hp2[:].rearrange("p (m b two) -> p m b two", m=HK, two=2),
                phv.unsqueeze(3).broadcast_to([128, HK, bc, 2]),
            )
            hp2v = hp2[:].rearrange("p (m b two) -> p m b two", m=HK, two=2)

        # e scores: per-k matmuls into separate psum blocks (no accumulation
        # groups -> each runs right after its tanh), DVE tree-sum at the end
        pe2 = ps_mix.tile([128, HK * nj], F32, tag="mix", name="pe2")
        for k in range(HK):
            sl = hproj[c][:, k * bt : (k + 1) * bt]
            if s == 0:
                th = pw.tile([128, bt], BF16, tag=f"th{c}", bufs=4)
                nc.scalar.activation(th[:], sl, AF.Tanh)
            else:
                a = pw.tile([128, bt], BF16, tag=f"a{c}", bufs=2)
                nc.vector.tensor_add(
                    a[:].rearrange("p (b t2 two) -> p b t2 two", b=bc, two=2),
                    sl.rearrange("p (b t2 two) -> p b t2 two", b=bc, two=2),
                    hp2v[:, k].unsqueeze(2).broadcast_to([128, bc, T // 2, 2]),
                )
                th = pw.tile([128, bt], BF16, tag=f"th{c}", bufs=4)
                nc.scalar.activation(th[:], a[:], AF.Tanh)
            for j in range(nj):
                nc.tensor.matmul(
                    pe2[:, k * nj + j : k * nj + j + 1],
                    th[:, 128 * j : 128 * j + 128],
                    w_sc[:, k : k + 1],
                    start=True,
                    stop=True,
                )
        e2 = psm.tile([128, nj], F32, tag="e2sb")
        nc.vector.reduce_sum(
            e2[:],
            pe2[:].rearrange("p (k j) -> p j k", k=HK),
            axis=mybir.AxisListType.X,
        )

        # transpose e to [nj, (2b x t)], softmax over t (fp32; no max-sub:
        # |e| <= ||w_score||_1 ~ 20 so fp32 exp is safe)
        ptr = ps_tr.tile([nj, 128], F32, tag="tr")
        nc.tensor.transpose(ptr[:], e2[:], id_f[:])
        ex = psm.tile([nj, 128], F32, tag="ex")
        nc.scalar.activation(ex[:], ptr[:], AF.Exp)
        ssum = psm.tile([nj, 2], F32, tag="ssum")
        nc.vector.reduce_sum(
            ssum[:], ex[:].rearrange("p (b t) -> p b t", b=2), axis=mybir.AxisListType.X
        )
        rinv = psm.tile([nj, 2], F32, tag="rinv")
        nc.vector.reciprocal(rinv[:], ssum[:])
        al = psm.tile([nj, 128], F32, tag="al")
        nc.vector.tensor_mul(
            al[:].rearrange("p (b t) -> p b t", b=2),
            ex[:].rearrange("p (b t) -> p b t", b=2),
            rinv[:].unsqueeze(2).broadcast_to([nj, 2, T]),
        )

        # alpha back to bt-partitions; build block-diag lhsT bands
        pac = ps_tr.tile([128, nj], F32, tag="tr")
        nc.tensor.transpose(pac[:], al[:], id_f[0:nj, 0:nj])
        adv = ad[c][:].rearrange("p (i two) -> p i two", two=2)
        for jj in range(2):
            nc.vector.tensor_copy(
                adv[64 * jj : 64 * jj + 64, :, jj], pac[64 * jj : 64 * jj + 64, :]
            )

        # ctxT[d, b] direct: lhsT = enc tile (d -> partitions), rhs = ad pair
        pctxT = ps_ctx.tile([128, HK * bc], F32, tag="ctxT_ps")
        for m in range(HK):
            for i in range(bc // 2):
                nc.tensor.matmul(
                    pctxT[:, m * bc + 2 * i : m * bc + 2 * i + 2],
                    enc_sb[c][:, 512 * i + 128 * m : 512 * i + 128 * m + 128],
                    ad[c][:, 2 * i : 2 * i + 2],
                    start=True,
                    stop=True,
                )
        nc.vector.tensor_copy(
            ctxT[:].rearrange("p (k b2) -> p k b2", k=HK)[:, :, c * bc : (c + 1) * bc],
            pctxT[:].rearrange("p (k b) -> p k b", k=HK),
        )

    for k in range(HK):
        pg = ps_g.tile([128, 4 * BW], F32, tag="g", name=f"pg{k}", bufs=4)
        for gi, gate in enumerate((0, 1, 3, 2)):
            m = 4 * gate + k
            col = pg[:, gi * BW : (gi + 1) * BW]
            for kk in range(HK):
                nc.tensor.matmul(
                    col,
                    w_hh[:, kk * G4 + 128 * m : kk * G4 + 128 * m + 128],
                    h_use[:, kk * BW : (kk + 1) * BW],
                    start=(kk == 0), stop=False, skip_group_check=True,
                )
            nc.tensor.matmul(
                col, w_oh[:, 128 * m : 128 * m + 128], ohsl,
                start=False, stop=False, skip_group_check=True,
            )
            for kk in range(HK):
                nc.tensor.matmul(
                    col,
                    w_ctx[:, kk * G4 + 128 * m : kk * G4 + 128 * m + 128],
                    ctxT[:, kk * BW : (kk + 1) * BW],
                    start=False, stop=(kk == HK - 1), skip_group_check=True,
                )
        tifo = psm.tile([128, 3 * BW], F32, tag="tifo")
        nc.scalar.activation(tifo[:], pg[:, 0 : 3 * BW], AF.Tanh, scale=0.5)
        tg = psm.tile([128, BW], F32, tag="tg")
        nc.scalar.activation(tg[:], pg[:, 3 * BW : 4 * BW], AF.Tanh)
        sifo = psm.tile([128, 3 * BW], F32, tag="sifo")
        nc.vector.tensor_scalar(sifo[:], tifo[:], 0.5, 0.5, ALU.mult, ALU.add)
        m1 = psm.tile([128, BW], F32, tag="m1")
        nc.vector.tensor_mul(m1[:], sifo[:, BW : 2 * BW], cT[:, k * BW : (k + 1) * BW])
        m2 = psm.tile([128, BW], F32, tag="m2")
        nc.vector.tensor_mul(m2[:], sifo[:, 0:BW], tg[:])
        nc.vector.tensor_add(cT[:, k * BW : (k + 1) * BW], m1[:], m2[:])
        tc_ = psm.tile([128, BW], F32, tag="tc")
        nc.scalar.activation(tc_[:], cT[:, k * BW : (k + 1) * BW], AF.Tanh)
        nc.vector.tensor_mul(
            hT[:, k * BW : (k + 1) * BW], sifo[:, 2 * BW : 3 * BW], tc_[:]
        )
        if s < steps - 1 and k % 2 == 1:
            # emit the (k-1, k) pair contiguously per m so psum groups are legal
            if k == 1:
                php_holder[0] = ps_mix.tile(
                    [128, 2 * HK * BW], F32, tag="mix", name="php"
                )
            blk = (k // 2) * HK * BW
            for m in range(HK):
                for kk in (k - 1, k):
                    nc.tensor.matmul(
                        php_holder[0][:, blk + m * BW : blk + (m + 1) * BW],
                        w_h2h[:, kk * H + 128 * m : kk * H + 128 * m + 128],
                        hT[:, kk * BW : (kk + 1) * BW],
                        start=(kk == k - 1),
                        stop=(kk == k),
                        skip_group_check=True,
                    )

    # -- probs = hT.T @ w_genT + b_gen -> DRAM --
    pp_ = ps_mix.tile([BW, C], F32, tag="mix", name="pp_")
    for k in range(HK):
        nc.tensor.matmul(
            pp_[:],
            hT[:, k * BW : (k + 1) * BW],
            w_gen[:, k * C : (k + 1) * C],
            start=(k == 0),
            stop=False,
            skip_group_check=True,
        )
    nc.tensor.matmul(
        pp_[:], ones[0:1, 0:BW], b_gen[:], start=False, stop=True, skip_group_check=True
    )
    po = psm.tile([BW, C], F32, tag="po")
    nc.vector.tensor_copy(po[:], pp_[:])
    nc.sync.dma_start(d_out[:, s, :], po[:])


# ------------------------- host side -------------------------


def prep_inputs(encoder_output, text, w_i2h, w_h2h, b_h2h, w_score, w_ih, w_hh,
                b_ih, b_hh, w_gen, b_gen, steps=S, nchunk=NCHUNK):
    """Build per-core input maps (numpy only)."""
    bc = BCORE // nchunk
    bt = bc * T
    enc = np.asarray(encoder_output, np.float32)
    text = np.asarray(text)

    wid = {}
    wid["w_i2hT"] = _tile128(np.asarray(w_i2h, np.float32).T.astype(BF))
    wid["w_h2hT"] = _tile128(np.asarray(w_h2h, np.float32).T.astype(BF))
    wid["w_scoreT"] = _tile128(np.asarray(w_score, np.float32).reshape(H, 1).astype(BF))
    wid["w_ctxT"] = _tile128(np.asarray(w_ih, np.float32)[:, :D].T.astype(BF))
    wid["w_hhT"] = _tile128(np.asarray(w_hh, np.float32).T.astype(BF))
    woh = np.zeros((128, G4), BF)  # K padded to 128 so FWL kicks in
    woh[:C] = np.asarray(w_ih, np.float32)[:, D:].T.astype(BF)
    woh[C] = (np.asarray(b_ih, np.float32) + np.asarray(b_hh, np.float32)).astype(BF)
    wid["w_ohT"] = woh
    wid["w_genT"] = _tile128(np.asarray(w_gen, np.float32).T.astype(BF))
    wid["b_gen"] = np.asarray(b_gen, np.float32).reshape(1, C).astype(BF)
    wid["b_h2hT"] = np.ascontiguousarray(
        np.asarray(b_h2h, np.float32).reshape(HK, 128).T
    )
    wid["id_f32"] = np.eye(128, dtype=np.float32)
    wid["ones_row"] = np.ones((1, BCORE), BF)

    in_maps = []
    for core in range(NCORES):
        rows = slice(core * BCORE, (core + 1) * BCORE)
        ec = enc[rows]  # [64, T, D]
        enc_sb = np.zeros((nchunk, 128, (bt // 128) * 512), BF)
        encT_sb = np.zeros((nchunk, 128, HK * bt), BF)
        for c in range(nchunk):
            flat = ec[c * bc : (c + 1) * bc].reshape(bt, D)  # b-major (b,t) rows
            enc_sb[c] = _tile128(flat.astype(BF))
            encT_sb[c] = _tile128(np.ascontiguousarray(flat.T).astype(BF))
        oh = np.zeros((128, steps * BCORE), BF)
        tx = text[rows]  # [64, S]
        for s in range(steps):
            oh[tx[:, s].astype(np.int64), s * BCORE + np.arange(BCORE)] = 1.0
        oh[C] = 1.0
        m = dict(wid)
        m["enc_sb"] = enc_sb
        m["encT_sb"] = encT_sb
        m["ohT_sb"] = oh
        in_maps.append(m)
    return in_maps


_NC_CACHE = {}


def get_nc(steps=S, nchunk=NCHUNK):
    key = (steps, nchunk)
    if key not in _NC_CACHE:
        _NC_CACHE[key] = build_nc(steps, nchunk)
    return _NC_CACHE[key]


def run(inputs, steps=S, nchunk=NCHUNK, trace=False):
    nc = get_nc(steps, nchunk)
    in_maps = prep_inputs(**inputs, steps=steps, nchunk=nchunk)
    res = run_bass_kernel_spmd(nc, in_maps, list(range(NCORES)), trace=trace)
    out = np.concatenate([res.results[i]["probs"] for i in range(NCORES)], axis=0)
    return out.astype(np.float32), res


def kernel(**inputs):
    out, _ = run(inputs)
    return out


# revision 20
# speedup vs baseline: 1.5028x; 1.0474x over previous
"""Trainium2 Bass kernel: attention-LSTM decoder (nn_Attention_74698071212133).

Sharding: data-parallel over batch across 8 NeuronCores (64 rows each), weights
replicated.  Each core splits its 64 rows into NCHUNK chunks whose attention
phases pipeline against each other; the LSTM/gates phase is fused across chunks
(wider matmuls, N=64) since the recurrence joins there anyway.

Per-core, per step s (batch b=32/chunk, T=64, H=512, C=38):
  h_use = snapshot(hT)                         (DVE)
  hpT   = w_h2h @ h_use       [fused chunks]   (PE)
  per chunk: a = H_projT + bcast_t(hpT)        (DVE, bf16 2x, 2048-wide)
             th = tanh(a)                      (ACT)
             e  = w_score . th -> psum[bt,16]  (PE, lhsT=th slices)
             softmax via PE transpose -> alpha (PE/DVE/ACT, fp32)
             ctxT = enc.T @ alpha-blockdiag    (PE, direct [d, b] layout)
  gatesT = W[ctxT; oh; 1; h_use] [fused]       (PE, K=128-padded onehots)
  lstm elementwise, sigmoid via 0.5*tanh(x/2)+0.5  (ACT+DVE)
  probs = hT.T @ w_genT + b_gen -> DRAM        (PE + DMA)
"""

import sys

sys.path.insert(0, "/opt/trn_rl_repo")

import numpy as np
import ml_dtypes

import concourse.bass as bass
import concourse.mybir as mybir
import concourse.tile as tile
from concourse import bacc
from concourse.bass_utils import run_bass_kernel_spmd

BF = ml_dtypes.bfloat16
F32 = mybir.dt.float32
BF16 = mybir.dt.bfloat16
AF = mybir.ActivationFunctionType
ALU = mybir.AluOpType

# Problem constants
B, T, D, H, C, S = 512, 64, 512, 512, 38, 26
NCORES = 8
BCORE = B // NCORES  # 64
NCHUNK = 2
G4 = 4 * H  # 2048
HK = H // 128  # 4 h-tiles


def _tile128(a):
    """[R, N] with R = r*128 -> [128, r*N] col-block layout (block k = rows 128k..)."""
    r = a.shape[0] // 128
    return np.ascontiguousarray(
        a.reshape(r, 128, a.shape[1]).transpose(1, 0, 2).reshape(128, -1)
    )


def build_nc(steps=S, nchunk=NCHUNK):
    bc = BCORE // nchunk  # batch per chunk
    bt = bc * T  # flattened (b, t) per chunk, b-major
    nbt = bt // 128  # 128-row bt tiles per chunk

    nc = bacc.Bacc()
    dp = nc.declare_dram_parameter
    # Per-core tensors (pre-tiled on host into [128, cols] SBUF images)
    d_enc = dp("enc_sb", [nchunk, 128, nbt * 512], BF16, isOutput=False)
    d_encT = dp("encT_sb", [nchunk, 128, HK * bt], BF16, isOutput=False)
    d_oh = dp("ohT_sb", [128, steps * BCORE], BF16, isOutput=False)
    # Replicated weights
    d_wi2h = dp("w_i2hT", [128, HK * H], BF16, isOutput=False)
    d_wh2h = dp("w_h2hT", [128, HK * H], BF16, isOutput=False)
    d_wsc = dp("w_scoreT", [128, HK], BF16, isOutput=False)
    d_wctx = dp("w_ctxT", [128, HK * G4], BF16, isOutput=False)
    d_whh = dp("w_hhT", [128, HK * G4], BF16, isOutput=False)
    d_woh = dp("w_ohT", [128, G4], BF16, isOutput=False)
    d_wgen = dp("w_genT", [128, HK * C], BF16, isOutput=False)
    d_bgen = dp("b_gen", [1, C], BF16, isOutput=False)
    d_bh2h = dp("b_h2hT", [128, HK], F32, isOutput=False)
    d_idf = dp("id_f32", [128, 128], F32, isOutput=False)
    d_ones = dp("ones_row", [1, BCORE], BF16, isOutput=False)
    d_out = dp("probs", [BCORE, steps, C], F32, isOutput=True)

    with tile.TileContext(nc) as tc:
        with (
            tc.tile_pool(name="consts", bufs=1) as pc,
            tc.tile_pool(name="persist", bufs=1) as pp,
        ):
            # ---- load constants ----
            def cload(dram, shape, dt):
                t_ = pc.tile(list(shape), dt, name=dram.tensor.name + "_sb")
                nc.sync.dma_start(t_[:], dram)
                return t_

            w_i2h = cload(d_wi2h[:], [128, HK * H], BF16)
            w_h2h = cload(d_wh2h[:], [128, HK * H], BF16)
            w_sc = cload(d_wsc[:], [128, HK], BF16)
            w_ctx = cload(d_wctx[:], [128, HK * G4], BF16)
            w_hh = cload(d_whh[:], [128, HK * G4], BF16)
            w_oh = cload(d_woh[:], [128, G4], BF16)
            w_gen = cload(d_wgen[:], [128, HK * C], BF16)
            b_gen = cload(d_bgen[:], [1, C], BF16)
            b_h2h = cload(d_bh2h[:], [128, HK], F32)
            id_f = cload(d_idf[:], [128, 128], F32)
            ones = cload(d_ones[:], [1, BCORE], BF16)
            ohT = cload(d_oh[:], [128, steps * BCORE], BF16)

            # ---- persistent state (fused layout: col-block k is BCORE wide,
            #      [chunk0 bc | chunk1 bc]) ----
            hT = pp.tile([128, HK * BCORE], BF16, tag="hT")
            cT = pp.tile([128, HK * BCORE], F32, tag="cT")
            ctxT = pp.tile([128, HK * BCORE], BF16, tag="ctxT")
            nc.vector.memset(hT[:], 0.0)
            nc.vector.memset(cT[:], 0.0)

            enc_sb, hproj, ad = [], [], []
            for c in range(nchunk):
                e_ = pp.tile([128, nbt * 512], BF16, tag=f"enc{c}")
                for q in range(4):
                    w = nbt * 512 // 4
                    nc.sync.dma_start(
                        e_[:, q * w : (q + 1) * w], d_enc[c, :, q * w : (q + 1) * w]
                    )
                enc_sb.append(e_)
                hproj.append(
                    pp.tile([128, HK * bt], BF16, tag=f"hproj{c}", name=f"hproj{c}")
                )
                a_ = pp.tile([128, bc], BF16, tag=f"ad{c}", name=f"ad{c}")
                nc.vector.memset(a_[:], 0.0)
                ad.append(a_)

            # ---- init: H_projT = w_i2h @ encT + b_h2h ----
            # encT pool is scoped: its recycled addresses give later writers
            # WAW deps on the input DMA queues, but bacc's event-semaphore
            # pass legalizes the wait fan-in.
            with (
                tc.tile_pool(name="encT", bufs=1) as pet,
                tc.tile_pool(name="initps", bufs=4, space="PSUM") as pips,
            ):
                for c in range(nchunk):
                    et = pet.tile([128, HK * bt], BF16, tag=f"encT{c}", name=f"encT{c}")
                    for q in range(4):
                        w = HK * bt // 4
                        nc.sync.dma_start(
                            et[:, q * w : (q + 1) * w],
                            d_encT[c, :, q * w : (q + 1) * w],
                        )
                    for m in range(HK):
                        for n in range(bt // 512):
                            ps = pips.tile([128, 512], F32, tag="initp")
                            for k in range(HK):
                                nc.tensor.matmul(
                                    ps[:],
                                    w_i2h[:, k * H + 128 * m : k * H + 128 * m + 128],
                                    et[:, k * bt + 512 * n : k * bt + 512 * n + 512],
                                    start=(k == 0),
                                    stop=(k == HK - 1),
                                )
                            nc.scalar.activation(
                                hproj[c][:, m * bt + 512 * n : m * bt + 512 * n + 512],
                                ps[:],
                                AF.Identity,
                                bias=b_h2h[:, m : m + 1],
                            )

            # ---- decode steps ----
            with (
                tc.tile_pool(name="work", bufs=6) as pw,
                tc.tile_pool(name="small", bufs=4) as psm,
                tc.tile_pool(name="ps_mix", bufs=2, space="PSUM") as ps_mix,
                tc.tile_pool(name="ps_tr", bufs=1, space="PSUM") as ps_tr,
                tc.tile_pool(name="ps_ctx", bufs=1, space="PSUM") as ps_ctx,
                tc.tile_pool(name="ps_g", bufs=4, space="PSUM") as ps_g,
            ):
                php_holder = [None]
                for s in range(steps):
                    step_body(
                        nc, s, steps, nchunk, bc, bt,
                        pw, psm, ps_mix, ps_tr, ps_ctx, ps_g,
                        enc_sb, hproj, hT, cT, ctxT, ad,
                        w_h2h, w_sc, w_ctx, w_hh, w_oh, w_gen, b_gen,
                        ohT, ones, id_f, d_out, php_holder,
                    )
    if not nc.is_finalized():
        nc.finalize()
    return nc


def step_body(
    nc, s, steps, nchunk, bc, bt,
    pw, psm, ps_mix, ps_tr, ps_ctx, ps_g,
    enc_sb, hproj, hT, cT, ctxT, ad,
    w_h2h, w_sc, w_ctx, w_hh, w_oh, w_gen, b_gen,
    ohT, ones, id_f, d_out, php_holder,
):
    nj = bt // 128
    BW = nchunk * bc  # fused col-block width (BCORE)
    php = php_holder[0]  # hp psum computed during the previous step's LSTM

    # -- gates, h + onehot contributions: emitted first so they are ready as
    #    soon as the previous LSTM tail finishes (reads of hT precede this
    #    step's writes in trace order, so WAR tracking keeps them correct) --
    ohsl = ohT[:, s * BW : (s + 1) * BW]
    pgs = []
    for k in range(HK):
        pg = ps_g.tile([128, 4 * BW], F32, tag="g", name=f"pg{k}", bufs=4)
        pgs.append(pg)
        # exactly ONE start=True per psum bank per step: start marks the whole
        # 2KB zero-region pending, so a second start would wipe other columns'
        # partial sums on their next write
        for gi, gate in enumerate((0, 1, 3, 2)):  # cols = [i, f, o, g]
            m = 4 * gate + k
            col = pg[:, gi * BW : (gi + 1) * BW]
            for kk in range(HK):
                nc.tensor.matmul(
                    col,
                    w_hh[:, kk * G4 + 128 * m : kk * G4 + 128 * m + 128],
                    hT[:, kk * BW : (kk + 1) * BW],
                    start=(gi == 0 and kk == 0),
                    stop=False,
                    skip_group_check=True,
                )
            nc.tensor.matmul(
                col, w_oh[:, 128 * m : 128 * m + 128], ohsl,
                start=False, stop=False, skip_group_check=True,
            )

    # -- attention per chunk (these pipeline against each other) --
    if s > 0:
        hps = psm.tile([128, HK * BW], F32, tag="hps")
        phv3 = php[:].rearrange("p (k x) -> p x k", k=2)
        for m in range(HK):
            nc.vector.reduce_sum(
                hps[:, m * BW : (m + 1) * BW],
                phv3[:, m * BW : (m + 1) * BW, :],
                axis=mybir.AxisListType.X,
            )
    for c in range(nchunk):
        if s > 0:
            # duplicate-x2 hp copies (enable DVE 2x mode on the broadcast add)
            hp2 = psm.tile([128, HK * bc * 2], BF16, tag=f"hp2_{c}")
            hp2v = hp2[:].rearrange("p (m b two) -> p m b two", m=HK, two=2)
            for m in range(HK):
                nc.vector.tensor_copy(
                    hp2v[:, m],
                    hps[:, m * BW + c * bc : m * BW + (c + 1) * bc]
                    .unsqueeze(2)
                    .broadcast_to([128, bc, 2]),
                )

        # e scores: per-k matmuls into separate psum blocks (no accumulation
        # groups -> each runs right after its tanh), DVE tree-sum at the end
        pe2 = ps_mix.tile([128, HK * nj], F32, tag="mix", name="pe2")
        for k in range(HK):
            sl = hproj[c][:, k * bt : (k + 1) * bt]
            if s == 0:
                th = pw.tile([128, bt], BF16, tag=f"th{c}", bufs=4)
                nc.scalar.activation(th[:], sl, AF.Tanh)
            else:
                a = pw.tile([128, bt], BF16, tag=f"a{c}", bufs=2)
                nc.vector.tensor_add(
                    a[:].rearrange("p (b t2 two) -> p b t2 two", b=bc, two=2),
                    sl.rearrange("p (b t2 two) -> p b t2 two", b=bc, two=2),
                    hp2v[:, k].unsqueeze(2).broadcast_to([128, bc, T // 2, 2]),
                )
                th = pw.tile([128, bt], BF16, tag=f"th{c}", bufs=4)
                nc.scalar.activation(th[:], a[:], AF.Tanh)
            for j in range(nj):
                nc.tensor.matmul(
                    pe2[:, k * nj + j : k * nj + j + 1],
                    th[:, 128 * j : 128 * j + 128],
                    w_sc[:, k : k + 1],
                    start=True,
                    stop=True,
                )
        e2 = psm.tile([128, nj], F32, tag="e2sb")
        nc.vector.reduce_sum(
            e2[:],
            pe2[:].rearrange("p (k j) -> p j k", k=HK),
            axis=mybir.AxisListType.X,
        )

        # transpose e to [nj, (2b x t)], softmax over t (fp32; no max-sub:
        # |e| <= ||w_score||_1 ~ 20 so fp32 exp is safe)
        ptr = ps_tr.tile([nj, 128], F32, tag="tr")
        nc.tensor.transpose(ptr[:], e2[:], id_f[:])
        ex = psm.tile([nj, 128], F32, tag="ex")
        nc.scalar.activation(ex[:], ptr[:], AF.Exp)
        ssum = psm.tile([nj, 2], F32, tag="ssum")
        nc.vector.reduce_sum(
            ssum[:], ex[:].rearrange("p (b t) -> p b t", b=2), axis=mybir.AxisListType.X
        )
        rinv = psm.tile([nj, 2], F32, tag="rinv")
        nc.vector.reciprocal(rinv[:], ssum[:])
        al = psm.tile([nj, 128], F32, tag="al")
        nc.vector.tensor_mul(
            al[:].rearrange("p (b t) -> p b t", b=2),
            ex[:].rearrange("p (b t) -> p b t", b=2),
            rinv[:].unsqueeze(2).broadcast_to([nj, 2, T]),
        )

        # alpha back to bt-partitions; build block-diag lhsT bands
        pac = ps_tr.tile([128, nj], F32, tag="tr")
        nc.tensor.transpose(pac[:], al[:], id_f[0:nj, 0:nj])
        adv = ad[c][:].rearrange("p (i two) -> p i two", two=2)
        for jj in range(2):
            nc.vector.tensor_copy(
                adv[64 * jj : 64 * jj + 64, :, jj], pac[64 * jj : 64 * jj + 64, :]
            )

        # ctxT[d, b] direct: lhsT = enc tile (d -> partitions), rhs = ad pair
        pctxT = ps_ctx.tile([128, HK * bc], F32, tag="ctxT_ps")
        for m in range(HK):
            for i in range(bc // 2):
                nc.tensor.matmul(
                    pctxT[:, m * bc + 2 * i : m * bc + 2 * i + 2],
                    enc_sb[c][:, 512 * i + 128 * m : 512 * i + 128 * m + 128],
                    ad[c][:, 2 * i : 2 * i + 2],
                    start=True,
                    stop=True,
                )
        nc.vector.tensor_copy(
            ctxT[:].rearrange("p (k b2) -> p k b2", k=HK)[:, :, c * bc : (c + 1) * bc],
            pctxT[:].rearrange("p (k b) -> p k b", k=HK),
        )

    for k in range(HK):
        pg = pgs[k]
        for gi, gate in enumerate((0, 1, 3, 2)):
            m = 4 * gate + k
            col = pg[:, gi * BW : (gi + 1) * BW]
            for kk in range(HK):
                nc.tensor.matmul(
                    col,
                    w_ctx[:, kk * G4 + 128 * m : kk * G4 + 128 * m + 128],
                    ctxT[:, kk * BW : (kk + 1) * BW],
                    start=False,
                    stop=(gi == 3 and kk == HK - 1),
                    skip_group_check=True,
                )
        tifo = psm.tile([128, 3 * BW], F32, tag="tifo")
        nc.scalar.activation(tifo[:], pg[:, 0 : 3 * BW], AF.Tanh, scale=0.5)
        tg = psm.tile([128, BW], F32, tag="tg")
        nc.scalar.activation(tg[:], pg[:, 3 * BW : 4 * BW], AF.Tanh)
        sifo = psm.tile([128, 3 * BW], F32, tag="sifo")
        nc.vector.tensor_scalar(sifo[:], tifo[:], 0.5, 0.5, ALU.mult, ALU.add)
        m1 = psm.tile([128, BW], F32, tag="m1")
        nc.vector.tensor_mul(m1[:], sifo[:, BW : 2 * BW], cT[:, k * BW : (k + 1) * BW])
        m2 = psm.tile([128, BW], F32, tag="m2")
        nc.vector.tensor_mul(m2[:], sifo[:, 0:BW], tg[:])
        nc.vector.tensor_add(cT[:, k * BW : (k + 1) * BW], m1[:], m2[:])
        tc_ = psm.tile([128, BW], F32, tag="tc")
        nc.scalar.activation(tc_[:], cT[:, k * BW : (k + 1) * BW], AF.Tanh)
        nc.vector.tensor_mul(
            hT[:, k * BW : (k + 1) * BW], sifo[:, 2 * BW : 3 * BW], tc_[:]
        )
        if s < steps - 1 and k % 2 == 1:
            # emit the (k-1, k) pair contiguously per m so psum groups are legal
            if k == 1:
                php_holder[0] = ps_mix.tile(
                    [128, 2 * HK * BW], F32, tag="mix", name="php"
                )
            blk = (k // 2) * HK * BW
            for m in range(HK):
                for kk in (k - 1, k):
                    nc.tensor.matmul(
                        php_holder[0][:, blk + m * BW : blk + (m + 1) * BW],
                        w_h2h[:, kk * H + 128 * m : kk * H + 128 * m + 128],
                        hT[:, kk * BW : (kk + 1) * BW],
                        start=(kk == k - 1),
                        stop=(kk == k),
                        skip_group_check=True,
                    )

    # -- probs = hT.T @ w_genT + b_gen -> DRAM --
    pp_ = ps_mix.tile([BW, C], F32, tag="mix", name="pp_")
    for k in range(HK):
        nc.tensor.matmul(
            pp_[:],
            hT[:, k * BW : (k + 1) * BW],
            w_gen[:, k * C : (k + 1) * C],
            start=(k == 0),
            stop=False,
            skip_group_check=True,
        )
    nc.tensor.matmul(
        pp_[:], ones[0:1, 0:BW], b_gen[:], start=False, stop=True, skip_group_check=True
    )
    po = psm.tile([BW, C], F32, tag="po")
    nc.vector.tensor_copy(po[:], pp_[:])
    nc.sync.dma_start(d_out[:, s, :], po[:])


# ------------------------- host side -------------------------


def prep_inputs(encoder_output, text, w_i2h, w_h2h, b_h2h, w_score, w_ih, w_hh,
                b_ih, b_hh, w_gen, b_gen, steps=S, nchunk=NCHUNK):
    """Build per-core input maps (numpy only)."""
    bc = BCORE // nchunk
    bt = bc * T
    enc = np.asarray(encoder_output, np.float32)
    text = np.asarray(text)

    wid = {}
    wid["w_i2hT"] = _tile128(np.asarray(w_i2h, np.float32).T.astype(BF))
    wid["w_h2hT"] = _tile128(np.asarray(w_h2h, np.float32).T.astype(BF))
    wid["w_scoreT"] = _tile128(np.asarray(w_score, np.float32).reshape(H, 1).astype(BF))
    wid["w_ctxT"] = _tile128(np.asarray(w_ih, np.float32)[:, :D].T.astype(BF))
    wid["w_hhT"] = _tile128(np.asarray(w_hh, np.float32).T.astype(BF))
    woh = np.zeros((128, G4), BF)  # K padded to 128 so FWL kicks in
    woh[:C] = np.asarray(w_ih, np.float32)[:, D:].T.astype(BF)
    woh[C] = (np.asarray(b_ih, np.float32) + np.asarray(b_hh, np.float32)).astype(BF)
    wid["w_ohT"] = woh
    wid["w_genT"] = _tile128(np.asarray(w_gen, np.float32).T.astype(BF))
    wid["b_gen"] = np.asarray(b_gen, np.float32).reshape(1, C).astype(BF)
    wid["b_h2hT"] = np.ascontiguousarray(
        np.asarray(b_h2h, np.float32).reshape(HK, 128).T
    )
    wid["id_f32"] = np.eye(128, dtype=np.float32)
    wid["ones_row"] = np.ones((1, BCORE), BF)

    in_maps = []
    for core in range(NCORES):
        rows = slice(core * BCORE, (core + 1) * BCORE)
        ec = enc[rows]  # [64, T, D]
        enc_sb = np.zeros((nchunk, 128, (bt // 128) * 512), BF)
        encT_sb = np.zeros((nchunk, 128, HK * bt), BF)
        for c in range(nchunk):
            flat = ec[c * bc : (c + 1) * bc].reshape(bt, D)  # b-major (b,t) rows
            enc_sb[c] = _tile128(flat.astype(BF))
            encT_sb[c] = _tile128(np.ascontiguousarray(flat.T).astype(BF))
        oh = np.zeros((128, steps * BCORE), BF)
        tx = text[rows]  # [64, S]
        for s in range(steps):
            oh[tx[:, s].astype(np.int64), s * BCORE + np.arange(BCORE)] = 1.0
        oh[C] = 1.0
        m = dict(wid)
        m["enc_sb"] = enc_sb
        m["encT_sb"] = encT_sb
        m["ohT_sb"] = oh
        in_maps.append(m)
    return in_maps


_NC_CACHE = {}


def get_nc(steps=S, nchunk=NCHUNK):
    key = (steps, nchunk)
    if key not in _NC_CACHE:
        _NC_CACHE[key] = build_nc(steps, nchunk)
    return _NC_CACHE[key]


def run(inputs, steps=S, nchunk=NCHUNK, trace=False):
    nc = get_nc(steps, nchunk)
    in_maps = prep_inputs(**inputs, steps=steps, nchunk=nchunk)
    res = run_bass_kernel_spmd(nc, in_maps, list(range(NCORES)), trace=trace)
    out = np.concatenate([res.results[i]["probs"] for i in range(NCORES)], axis=0)
    return out.astype(np.float32), res


def kernel(**inputs):
    out, _ = run(inputs)
    return out
